# revision 6
# baseline (speedup 1.0000x reference)
"""Trainium2 Bass kernel for nn_Diffusion_Model (ragged_sequence).

Pure data-parallel: batch B=4096 sharded as 512 per NeuronCore across 8 cores.

Fast path (staged weights): the ragged tail is degenerate (Ln == 64 for all
(b, n)) and |q-1| <= 0.0055, so the geometric tail sum_t q^(63-t) u_t is a
degree-4 polynomial in delta = q-1 whose coefficient sums G_k = sum_t
C(63-t,k) u_t come out of the PE as 5 extra (static-weight) matmul columns.
The relu dot uses sum W12 relu(h) = (d0 + sum W12|h|)/2 with d0 as one more
matmul column. No transpose, no scan, no per-token masking. bf16 input DMA +
bf16 matmuls; Pool does the squares and |h|*W12 products; DVE only pair-adds
and the two segmented reduces; per-(b,n) scalar chain runs batch-wide once.

Fallback (general weights): the previous scan-based kernel (PE transpose +
DVE Horner scan, tensor_mask_reduce extraction for ragged Ln).
"""
import sys
import numpy as np

sys.path.insert(0, "/opt/trn_rl_repo")

import concourse.bacc as bacc
import concourse.tile as tile
import concourse.mybir as mybir
from concourse.bass_utils import run_bass_kernel_spmd

dt = mybir.dt
Alu = mybir.AluOpType
Act = mybir.ActivationFunctionType
Ax = mybir.AxisListType

N_CORES = 8
B, T, N, H = 4096, 64, 128, 64
LN_EPS = 1e-5


def _np32(a):
    return np.ascontiguousarray(np.asarray(a, dtype=np.float32))


class _Built:
    pass


NPOLY = 5          # delta powers 0..4
GB = 16            # batches per group
NCOL = NPOLY + 1   # d0 column + NPOLY G columns


def poly_gate(w):
    """True iff the polynomial tail is valid: Ln==64 everywhere and
    63*|q-1| small enough for a degree-4 expansion."""
    lo = w["b3"] + 1e-5 + min(w["w30"], 0.0) + min(w["w31"], 0.0)
    hi = w["b3"] + 1e-5 + max(w["w30"], 0.0) + max(w["w31"], 0.0)
    m = 1e-3
    degenerate = (lo > -10.0 + m) and (hi < -m) and w["alpha"] >= 0.0
    if not degenerate:
        return False
    a50 = 50.0 * w["alpha"]
    d_lo, d_hi = a50 + lo, a50 + hi
    if d_lo <= 1.0:
        return False
    dmax = max(abs(a50 / d_lo - 1.0), abs(a50 / d_hi - 1.0))
    return 63.0 * dmax <= 0.5


def _bf16(a):
    return np.asarray(a, dtype=np.float32).astype(mybir.dt.np(dt.bfloat16))


# Per-pair engine assignment for the elementwise stages (16 pairs of 2
# groups).  'D' = DVE, 'P' = gpsimd/Pool, 'A' = ACT (sq only).
SQ_ENG = ['A' if p % 3 == 2 else 'D' for p in range(16)]
MDW_ENG = ['D'] * 16
PA_ENG = ['P' if p % 3 == 1 else 'D' for p in range(16)]
PB_ENG = ['P' if p % 2 == 0 else 'D' for p in range(16)]
PC_ENG = ['P' if p % 2 == 1 else 'D' for p in range(16)]
PD_ENG = ['P' if p % 2 == 0 else 'D' for p in range(16)]
HORNER_ENG = ['P', 'D', 'D']


def build_poly(weights, BPC, unroll=1, hwloop=None, variant="full"):
    NG = BPC // GB                    # 32 groups
    NPAIR = NG // 2                   # 16 pairs
    CHUNK_GROUPS = [12, 12, 8]        # tail chunks (groups per chunk)
    assert sum(CHUNK_GROUPS) == NG
    nc = bacc.Bacc("TRN2", target_bir_lowering=False, debug=False,
                   num_devices=N_CORES)

    x_in = nc.dram_tensor("x", [T + 1, BPC * (N + 1)], dt.bfloat16,
                          kind="ExternalInput").ap()
    d_in = nc.dram_tensor("d", [T + 1, BPC], dt.bfloat16,
                          kind="ExternalInput").ap()
    y_out = nc.dram_tensor("y", [BPC, 1], dt.float32,
                           kind="ExternalOutput").ap()

    W11, b11 = weights["W11"], weights["b11"]
    W21, b21 = weights["W21"], weights["b21"]
    W12, b12 = weights["W12"], float(weights["b12"])
    W22, b22 = weights["W22"], float(weights["b22"])
    w30, w31, b3 = (float(weights["w30"]), float(weights["w31"]),
                    float(weights["b3"]))
    alpha = float(weights["alpha"])
    a50 = 50.0 * alpha

    # LN mean-centering folded into the weights (exact linear algebra).
    W11c = (W11.astype(np.float64)
            - W11.astype(np.float64).mean(1, keepdims=True)).astype(np.float32)
    b11c = (b11.astype(np.float64) - b11.astype(np.float64).mean()).astype(np.float32)
    W21c = (W21.astype(np.float64)
            - W21.astype(np.float64).mean(1, keepdims=True)).astype(np.float32)
    b21c = (b21.astype(np.float64) - b21.astype(np.float64).mean()).astype(np.float32)

    RWm = np.zeros((T + 1, H), np.float32)
    RWm[:T] = W11c
    RWm[T] = b11c
    # G columns: col 0 = d0 (W11c @ W12/2, relu linear part); cols 1..5 =
    # NEGATED binomial tail weights C(63-t, k) so that accm = m*s equals
    # +F*poly and pred is a plain partition-sum (gpsimd C-reduce).
    RWG = np.zeros((T + 1, NCOL), np.float32)
    w12h = 0.5 * W12.astype(np.float64)
    RWG[:T, 0] = (W11c.astype(np.float64) @ w12h).astype(np.float32)
    RWG[T, 0] = float(b11c.astype(np.float64) @ w12h)
    from math import comb
    for k in range(NPOLY):
        RWG[:T, k + 1] = [-float(comb(63 - int(t), k)) for t in range(T)]

    RW2 = np.zeros((T + 1, H), np.float32)
    RW2[:T] = W21c
    RW2[T] = b21c

    w12bc = np.broadcast_to((0.5 * W12)[None, :], (128, H)).copy()
    w22_col = W22.reshape(T, 1).astype(np.float32)

    # pack all [65-row] bf16 constants into one tensor -> one DMA
    CPK = np.zeros((T + 1, H + NCOL + H + 2), np.float32)
    CPK[:, 0:H] = RWm
    CPK[:, H:H + NCOL] = RWG
    CPK[:, H + NCOL:2 * H + NCOL] = RW2
    CPK[:T, 2 * H + NCOL] = 1.0
    CPK[:T, 2 * H + NCOL + 1] = w22_col[:, 0]
    c_CPK = nc.inline_tensor(_bf16(CPK), "c_CPK")
    c_w12bc = nc.inline_tensor(_bf16(w12bc), "c_w12bc")

    def _chunks(total, step=512):
        return [slice(i, min(i + step, total)) for i in range(0, total, step)]

    with tile.TileContext(nc) as tc:
        with tc.tile_pool(name="const", bufs=1) as cpool:
            CPK_sb = cpool.tile([T + 1, H + NCOL + H + 2], dt.bfloat16)
            nc.sync.dma_start(CPK_sb[:], c_CPK.ap())
            RWm_sb = CPK_sb[:, 0:H]
            RWG_sb = CPK_sb[:, H:H + NCOL]
            RW2_sb = CPK_sb[:, H + NCOL:2 * H + NCOL]
            ones64_sb = CPK_sb[0:T, 2 * H + NCOL:2 * H + NCOL + 1]
            w22_sb = CPK_sb[0:T, 2 * H + NCOL + 1:2 * H + NCOL + 2]
            w12_sb = cpool.tile([128, H], dt.bfloat16)
            nc.sync.dma_start(w12_sb[:], c_w12bc.ap())

            eps_sb = cpool.tile([128, 1], dt.float32)
            nc.vector.memset(eps_sb[:], LN_EPS)
            one_sb = cpool.tile([128, 1], dt.float32)
            nc.vector.memset(one_sb[:], 1.0)
            nb12_sb = cpool.tile([128, 1], dt.float32)
            nc.vector.memset(nb12_sb[:], -b12)
            nb22_sb = cpool.tile([128, 1], dt.float32)
            nc.vector.memset(nb22_sb[:], -b22)
            onesrow_sb = cpool.tile([1, 128], dt.float32)
            nc.vector.memset(onesrow_sb[:], 1.0)

            with tc.tile_pool(name="acc", bufs=2) as apool, \
                 tc.tile_pool(name="b2", bufs=2) as b2, \
                 tc.tile_pool(name="b2ps", bufs=1, space="PSUM") as b2ps, \
                 tc.tile_pool(name="xt", bufs=6) as xpool, \
                 tc.tile_pool(name="psh", bufs=2, space="PSUM") as pshp, \
                 tc.tile_pool(name="psg", bufs=2, space="PSUM") as psgp, \
                 tc.tile_pool(name="habsp", bufs=3) as habsp, \
                 tc.tile_pool(name="mdp", bufs=3) as mdp, \
                 tc.tile_pool(name="pap", bufs=3) as papool, \
                 tc.tile_pool(name="pbp", bufs=3) as pbpool, \
                 tc.tile_pool(name="tail", bufs=2) as tp:
                import contextlib
                loop_ctxs = []
                if hwloop is not None and hwloop > 1:
                    unroll = 1
                for rep in range(unroll):
                    ctx = (tc.For_i(0, hwloop, 1,
                                    hint_engines=(mybir.EngineType.PE,),
                                    name="reploop")
                           if hwloop is not None and hwloop > 1
                           else contextlib.nullcontext())
                    loop_ctxs.append(ctx)
                    ctx.__enter__()
                    CB_sb = apool.tile([128, BPC], dt.float32, tag="CB")
                    pred = apool.tile([1, BPC], dt.float32, tag="pred")

                    do_b2 = variant != "dma_only"
                    do_compute = variant != "dma_only"
                    # ---- branch 2 (down channel) -> CB = w31*xd + const,
                    # broadcast over partitions; overlaps the group loop ----
                    dstage = b2.tile([T + 1, BPC], dt.bfloat16, tag="dstage")
                    nc.sync.dma_start(dstage[:], d_in[:])
                    if do_b2:
                        ps2c = b2ps.tile([128, BPC], dt.float32, tag="big")
                        for sl in _chunks(BPC):
                            nc.tensor.matmul(ps2c[0:H, sl], RW2_sb,
                                             dstage[:, sl], start=True, stop=True)
                        sq2 = b2.tile([H, BPC], dt.bfloat16, tag="sq2")
                        nc.scalar.square(sq2[:], ps2c[0:H, :])
                        psS2 = b2ps.tile([128, BPC], dt.float32, tag="big")
                        for sl in _chunks(BPC):
                            nc.tensor.matmul(psS2[0:1, sl], ones64_sb,
                                             sq2[:, sl], start=True, stop=True)
                        rl2 = b2.tile([H, BPC], dt.bfloat16, tag="rl2")
                        nc.scalar.activation(rl2[:], ps2c[0:H, :], Act.Relu)
                        lnv2 = b2.tile([1, BPC], dt.float32, tag="lnv2")
                        nc.scalar.activation(lnv2[:], psS2[0:1, :], Act.Ln,
                                             scale=1.0 / H, bias=eps_sb[0:1, :])
                        rstd2 = b2.tile([1, BPC], dt.float32, tag="rstd2")
                        nc.scalar.activation(rstd2[:], lnv2[:], Act.Exp,
                                             scale=-0.5)
                        psD = b2ps.tile([128, BPC], dt.float32, tag="big")
                        for sl in _chunks(BPC):
                            nc.tensor.matmul(psD[0:1, sl], w22_sb, rl2[:, sl],
                                             start=True, stop=True)
                        xin2 = b2.tile([1, BPC], dt.float32, tag="xin2")
                        nc.vector.tensor_tensor(xin2[:], psD[0:1, :], rstd2[:],
                                                Alu.mult)
                        en2 = b2.tile([1, BPC], dt.float32, tag="en2")
                        nc.scalar.activation(en2[:], xin2[:], Act.Exp,
                                             scale=-1.0, bias=nb22_sb[0:1, :])
                        nc.vector.tensor_scalar(en2[:], en2[:], 1.0, None,
                                                Alu.add)
                        xdrow = b2.tile([1, BPC], dt.float32, tag="xdrow")
                        nc.vector.reciprocal(xdrow[:], en2[:])
                        # fold CB = w31*xd + (b3 + 1e-5 + 50*alpha) pre-broadcast
                        crow = b2.tile([1, BPC], dt.float32, tag="crow")
                        nc.gpsimd.tensor_scalar(crow[:], xdrow[:], w31,
                                                b3 + 1e-5 + a50, Alu.mult,
                                                Alu.add)
                        psC = b2ps.tile([128, BPC], dt.float32, tag="big")
                        for sl in _chunks(BPC):
                            nc.tensor.matmul(psC[:, sl], onesrow_sb[:],
                                             crow[:, sl], start=True, stop=True)
                        nc.scalar.copy(CB_sb[:], psC[:])

                    # ---- main loop: chunks x pairs of groups of GB ----
                    w12v = w12_sb[:].unsqueeze(1).broadcast_to([128, 2 * GB, H])
                    if variant == "nodma":
                        xt_static = xpool.tile([T + 1, 2 * GB * (N + 1)],
                                               dt.bfloat16, tag="xts")
                        nc.sync.dma_start(
                            xt_static[:], x_in[:, 0:2 * GB * (N + 1)])
                    g0 = 0
                    for c, GPC in enumerate(CHUNK_GROUPS):
                        CW = GPC * GB
                        comb_c = tp.tile([128, 2 * CW], dt.float32,
                                         tag=f"comb{c}")
                        G_c = tp.tile([128, CW * NCOL], dt.bfloat16,
                                      tag=f"G{c}")
                        for pp in range(GPC // 2):
                            pg = (g0 + 2 * pp) // 2   # global pair idx
                            habs2 = habsp.tile([128, 2 * GB * H], dt.bfloat16,
                                               tag="habs2")
                            for half in range(2):
                                g = g0 + 2 * pp + half
                                if variant == "nodma":
                                    xt = xt_static
                                elif g % 2 == 0:
                                    xt = xpool.tile(
                                        [T + 1, 2 * GB * (N + 1)],
                                        dt.bfloat16)
                                    nc.sync.dma_start(
                                        xt[:],
                                        x_in[:, g * GB * (N + 1):
                                             (g + 2) * GB * (N + 1)])
                                xoff = (g % 2) * GB * (N + 1)
                                if not do_compute:
                                    continue
                                ps_h = pshp.tile([128, GB * H], dt.float32)
                                if half == 0:
                                    ps_g2 = psgp.tile(
                                        [128, 2 * GB * NCOL], dt.float32)
                                ps_g = ps_g2[:, half * GB * NCOL:
                                             (half + 1) * GB * NCOL]
                                for j in range(GB):
                                    lhs = xt[:, xoff + j * (N + 1):
                                             xoff + j * (N + 1) + N]
                                    nc.tensor.matmul(
                                        ps_h[:, j * H:(j + 1) * H], lhs,
                                        RWm_sb, start=True, stop=True)
                                    nc.tensor.matmul(
                                        ps_g[:, j * NCOL:(j + 1) * NCOL], lhs,
                                        RWG_sb, start=True, stop=True)
                                nc.scalar.activation(
                                    habs2[:, half * GB * H:(half + 1) * GB * H],
                                    ps_h[:], Act.Abs)
                            if do_compute:
                                gg0 = 2 * pp
                                nc.scalar.copy(
                                    G_c[:, gg0 * GB * NCOL:
                                        (gg0 + 2) * GB * NCOL],
                                    ps_g2[:])
                            if not do_compute:
                                continue
                            # ---- paired elementwise stages ----
                            PW = 2 * GB * H          # 2048
                            md = mdp.tile([128, 2 * PW], dt.bfloat16,
                                          tag="md")
                            sq_e = SQ_ENG[pg]
                            if sq_e == 'A':
                                nc.scalar.activation(md[:, 0:PW], habs2[:],
                                                     Act.Square)
                            else:
                                eng = nc.vector if sq_e == 'D' else nc.gpsimd
                                eng.tensor_tensor(md[:, 0:PW], habs2[:],
                                                  habs2[:], Alu.mult)
                            eng = nc.vector if MDW_ENG[pg] == 'D' else nc.gpsimd
                            eng.tensor_tensor(
                                md[:, PW:2 * PW].rearrange(
                                    "p (g t) -> p g t", g=2 * GB),
                                habs2[:].rearrange("p (g t) -> p g t",
                                                   g=2 * GB),
                                w12v, Alu.mult)
                            md3 = md[:].rearrange(
                                "p (two b t) -> p two b t", two=2, b=2 * GB)
                            pa = papool.tile([128, PW], dt.bfloat16, tag="pa")
                            pa3 = pa[:].rearrange(
                                "p (two b t) -> p two b t", two=2, b=2 * GB)
                            eng = nc.vector if PA_ENG[pg] == 'D' else nc.gpsimd
                            eng.tensor_tensor(pa3, md3[:, :, :, 0:H // 2],
                                              md3[:, :, :, H // 2:H], Alu.add)
                            pb = pbpool.tile([128, PW // 2], dt.bfloat16,
                                             tag="pb")
                            pb3 = pb[:].rearrange(
                                "p (two b t) -> p two b t", two=2, b=2 * GB)
                            eng = nc.vector if PB_ENG[pg] == 'D' else nc.gpsimd
                            eng.tensor_tensor(pb3, pa3[:, :, :, 0:H // 4],
                                              pa3[:, :, :, H // 4:H // 2],
                                              Alu.add)
                            pc = pbpool.tile([128, PW // 4], dt.bfloat16,
                                             tag="pc")
                            pc3 = pc[:].rearrange(
                                "p (two b t) -> p two b t", two=2, b=2 * GB)
                            eng = nc.vector if PC_ENG[pg] == 'D' else nc.gpsimd
                            eng.tensor_tensor(pc3, pb3[:, :, :, 0:H // 8],
                                              pb3[:, :, :, H // 8:H // 4],
                                              Alu.add)
                            pd = pbpool.tile([128, PW // 8], dt.bfloat16,
                                             tag="pd")
                            pd3 = pd[:].rearrange(
                                "p (two b t) -> p two b t", two=2, b=2 * GB)
                            eng = nc.vector if PD_ENG[pg] == 'D' else nc.gpsimd
                            eng.tensor_tensor(pd3, pc3[:, :, :, 0:H // 16],
                                              pc3[:, :, :, H // 16:H // 8],
                                              Alu.add)
                            poff = 2 * pp * GB
                            nc.vector.tensor_reduce(
                                comb_c[:].rearrange(
                                    "p (two b) -> p two b",
                                    two=2)[:, :, poff:poff + 2 * GB],
                                pd3, Ax.X, Alu.add)

                        # ---- tail for this chunk of CW batches ----
                        csl = slice(g0 * GB, g0 * GB + CW)
                        g0 += GPC
                        if not do_compute:
                            continue
                        Gv = G_c[:].rearrange("p (b k) -> p b k", k=NCOL)
                        sqv = comb_c[:, 0:CW]
                        d1v = comb_c[:, CW:2 * CW]
                        lnv = tp.tile([128, CW], dt.float32, tag="lnv")
                        nc.scalar.activation(lnv[:], sqv, Act.Ln,
                                             scale=1.0 / H, bias=eps_sb[:])
                        rstd = tp.tile([128, CW], dt.float32, tag="rstd")
                        nc.scalar.activation(rstd[:], lnv[:], Act.Exp,
                                             scale=-0.5)
                        dot = tp.tile([128, CW], dt.float32, tag="dot")
                        nc.vector.tensor_tensor(dot[:], d1v, Gv[:, :, 0],
                                                Alu.add)
                        xin = tp.tile([128, CW], dt.float32, tag="xin")
                        nc.vector.tensor_tensor(xin[:], dot[:], rstd[:],
                                                Alu.mult)
                        e = tp.tile([128, CW], dt.float32, tag="e")
                        nc.scalar.activation(e[:], xin[:], Act.Exp,
                                             scale=-1.0, bias=nb12_sb[:])
                        l1 = tp.tile([128, CW], dt.float32, tag="l1")
                        nc.scalar.activation(l1[:], e[:], Act.Ln, scale=1.0,
                                             bias=one_sb[:])
                        xu = tp.tile([128, CW], dt.float32, tag="xu")
                        nc.scalar.activation(xu[:], l1[:], Act.Exp,
                                             scale=-1.0)
                        D = tp.tile([128, CW], dt.float32, tag="D")
                        nc.vector.scalar_tensor_tensor(D[:], xu[:], w30,
                                                       CB_sb[:, csl],
                                                       Alu.mult, Alu.add)
                        lD = tp.tile([128, CW], dt.float32, tag="lD")
                        nc.scalar.activation(lD[:], D[:], Act.Ln,
                                             scale=1.0 / a50)
                        q = tp.tile([128, CW], dt.float32, tag="q")
                        nc.scalar.activation(q[:], lD[:], Act.Exp, scale=-1.0)
                        m = tp.tile([128, CW], dt.bfloat16, tag="m")
                        nc.vector.tensor_scalar(m[:], q[:], 1.0, None,
                                                Alu.subtract)
                        # Horner: s = G1 + m*(G2 + m*(G3 + m*(G4 + m*G5)))
                        heng = (nc.vector if HORNER_ENG[c % len(HORNER_ENG)]
                                == 'D' else nc.gpsimd)
                        t_ = tp.tile([128, CW], dt.bfloat16, tag="hm")
                        heng.tensor_tensor(t_[:], m[:], Gv[:, :, NPOLY],
                                           Alu.mult)
                        s = tp.tile([128, CW], dt.bfloat16, tag="hs")
                        heng.tensor_tensor(s[:], Gv[:, :, NPOLY - 1],
                                           t_[:], Alu.add)
                        for k in range(NPOLY - 2, 0, -1):
                            t_ = tp.tile([128, CW], dt.bfloat16, tag=f"hm{k}")
                            heng.tensor_tensor(t_[:], m[:], s[:], Alu.mult)
                            s = tp.tile([128, CW], dt.bfloat16, tag=f"hs{k}")
                            heng.tensor_tensor(s[:], Gv[:, :, k], t_[:],
                                               Alu.add)
                        accm = tp.tile([128, CW], dt.bfloat16, tag="accm")
                        heng.tensor_tensor(accm[:], m[:], s[:], Alu.mult)
                        # pred[b] = sum_n accm  (G negated: accm = +F*s)
                        nc.gpsimd.tensor_reduce(pred[0:1, csl], accm[:],
                                                Ax.C, Alu.add)

                    if not do_compute:
                        nc.vector.memset(pred[:], 0.0)
                    nc.sync.dma_start(
                        y_out.rearrange("b one -> one b"), pred[:])
                    loop_ctxs.pop().__exit__(None, None, None)

    # Pin every activation to the one table containing ln/exp/abs/relu/square
    # so the stream has a single ACT table load.
    import types
    from concourse.hw_specs import get_activation_tables
    orig_tables = list(get_activation_tables(nc.m.arch).items())
    patched_tables = [
        (name, s if name == "natural_log_exp_and_others" else set())
        for name, s in orig_tables
    ]
    import bass_rust as _bass_rust_mod

    def _patched_act_loads(self):
        has_activation = any(
            type(i).__name__ == "InstActivation"
            for b in self.main_func.blocks
            for i in b.instructions
        )
        if not has_activation:
            return
        _bass_rust_mod.insert_act_table_loads(self, patched_tables)

    nc.insert_act_table_loads = types.MethodType(_patched_act_loads, nc)

    nc.compile()

    built = _Built()
    built.nc = nc
    built.BPC = BPC
    return built


def _build_scan(weights, BPC, ln_affine, general_tail, unroll=1):
    """Build the per-core Bass program. weights: dict of host-folded arrays."""
    NG = BPC // 8  # groups of 8 batches
    nc = bacc.Bacc("TRN2", target_bir_lowering=False, debug=False,
                   num_devices=N_CORES)

    x_in = nc.dram_tensor("x", [T + 1, BPC, N + 1], dt.float32,
                          kind="ExternalInput").ap()
    d_in = nc.dram_tensor("d", [T + 1, BPC], dt.float32,
                          kind="ExternalInput").ap()
    y_out = nc.dram_tensor("y", [BPC, 1], dt.float32,
                           kind="ExternalOutput").ap()

    # ---- inline constants ----
    W11, b11 = weights["W11"], weights["b11"]     # [64,64],[64]
    W21, b21 = weights["W21"], weights["b21"]
    W12, b12 = weights["W12"], float(weights["b12"])   # [64], scalar
    W22, b22 = weights["W22"], float(weights["b22"])
    g11, be11 = weights["g11"], weights["be11"]
    g21, be21 = weights["g21"], weights["be21"]
    w30, w31, b3 = (float(weights["w30"]), float(weights["w31"]),
                    float(weights["b3"]))
    alpha = float(weights["alpha"])

    # fold LN mean-centering into the weights (exact linear algebra):
    # mean_j of (x@W + b) = x@rowmean(W) + mean(b); subtracting it is the
    # same matmul with row-centered W and mean-centered b.
    W11c = (W11.astype(np.float64)
            - W11.astype(np.float64).mean(1, keepdims=True)).astype(np.float32)
    b11c = (b11.astype(np.float64) - b11.astype(np.float64).mean()).astype(np.float32)
    W21c = (W21.astype(np.float64)
            - W21.astype(np.float64).mean(1, keepdims=True)).astype(np.float32)
    b21c = (b21.astype(np.float64) - b21.astype(np.float64).mean()).astype(np.float32)
    RW = np.zeros((T + 1, 2 * H), np.float32)
    RW[:T, :H] = W11c
    RW[T, :H] = b11c
    RW[:T, H:] = np.eye(T, dtype=np.float32)
    RW2 = np.zeros((T + 1, H), np.float32)
    RW2[:T, :] = W21c
    RW2[T, :] = b21c

    w12bc = np.broadcast_to(W12[None, :], (128, H)).copy()
    g11bc = np.broadcast_to(g11[None, :], (128, H)).copy()
    be11bc = np.broadcast_to(be11[None, :], (128, H)).copy()
    t0m = np.ones((128, 512), np.float32)
    t0m[:, ::T] = 0.0
    ones_col = np.ones((128, 1), np.float32)
    ones64_col = np.ones((T, 1), np.float32)
    noneg64_row = np.full((1, H), -1.0 / H, np.float32)
    w22_col = W22.reshape(T, 1).astype(np.float32)

    c_RW = nc.inline_tensor(RW, "c_RW")
    c_RW2 = nc.inline_tensor(RW2, "c_RW2")
    c_w12bc = nc.inline_tensor(w12bc, "c_w12bc")
    c_t0m = nc.inline_tensor(t0m, "c_t0m")
    c_ones = nc.inline_tensor(ones_col, "c_ones")
    c_ones64 = nc.inline_tensor(ones64_col, "c_ones64")
    c_negmean = nc.inline_tensor(noneg64_row, "c_negmean")
    c_w22 = nc.inline_tensor(w22_col, "c_w22")
    if ln_affine:
        c_g11bc = nc.inline_tensor(g11bc, "c_g11bc")
        c_be11bc = nc.inline_tensor(be11bc, "c_be11bc")

    def _chunks(total, step=512):
        return [slice(i, min(i + step, total)) for i in range(0, total, step)]

    with tile.TileContext(nc) as tc:
        with tc.tile_pool(name="const", bufs=1) as cpool:
            RW_sb = cpool.tile([T + 1, 2 * H], dt.float32)
            nc.sync.dma_start(RW_sb[:], c_RW.ap())
            RW2_sb = cpool.tile([T + 1, H], dt.float32)
            nc.sync.dma_start(RW2_sb[:], c_RW2.ap())
            w12_sb = cpool.tile([128, H], dt.float32)
            nc.sync.dma_start(w12_sb[:], c_w12bc.ap())
            t0m_sb = cpool.tile([128, 512], dt.float32)
            nc.sync.dma_start(t0m_sb[:], c_t0m.ap())
            ones_sb = cpool.tile([128, 1], dt.float32)
            nc.sync.dma_start(ones_sb[:], c_ones.ap())
            ones64_sb = cpool.tile([T, 1], dt.float32)
            nc.sync.dma_start(ones64_sb[:], c_ones64.ap())
            negmean_sb = cpool.tile([1, H], dt.float32)
            nc.sync.dma_start(negmean_sb[:], c_negmean.ap())
            w22_sb = cpool.tile([T, 1], dt.float32)
            nc.sync.dma_start(w22_sb[:], c_w22.ap())
            if ln_affine:
                g11_sb = cpool.tile([128, H], dt.float32)
                nc.sync.dma_start(g11_sb[:], c_g11bc.ap())
                be11_sb = cpool.tile([128, H], dt.float32)
                nc.sync.dma_start(be11_sb[:], c_be11bc.ap())

            eps_sb = cpool.tile([128, 1], dt.float32)
            nc.vector.memset(eps_sb[:], LN_EPS)
            b22_sb = cpool.tile([128, 1], dt.float32)
            nc.vector.memset(b22_sb[:], b22)
            nb12_sb = cpool.tile([128, 1], dt.float32)
            nc.vector.memset(nb12_sb[:], -b12)
            nb22_sb = cpool.tile([128, 1], dt.float32)
            nc.vector.memset(nb22_sb[:], -b22)

            xdbc_sb = cpool.tile([128, BPC], dt.float32)   # xd broadcast
            acc_sb = cpool.tile([128, BPC], dt.float32)    # F * s* per token

            # ================= branch 2 (down channel), once =================
            with tc.tile_pool(name="b2", bufs=1) as b2, \
                 tc.tile_pool(name="b2ps", bufs=1, space="PSUM") as b2ps:
                onesrow = b2.tile([1, 128], dt.float32)
                nc.vector.memset(onesrow[:], 1.0)
                dstage = b2.tile([T + 1, BPC], dt.float32)
                nc.sync.dma_start(dstage[:], d_in[:])
                ps2c = b2ps.tile([H, BPC], dt.float32)
                for sl in _chunks(BPC):
                    nc.tensor.matmul(ps2c[:, sl], RW2_sb[:], dstage[:, sl],
                                     start=True, stop=True)
                sq2 = b2.tile([H, BPC], dt.float32)
                nc.scalar.square(sq2[:], ps2c[:])
                psS2 = b2ps.tile([1, BPC], dt.float32)
                for sl in _chunks(BPC):
                    nc.tensor.matmul(psS2[0:1, sl], ones64_sb, sq2[:, sl],
                                     start=True, stop=True)
                rl2 = b2.tile([H, BPC], dt.float32)
                if ln_affine:
                    # general gamma/beta for branch 2: nh*g+be then relu
                    lnv2r = b2.tile([1, BPC], dt.float32)
                    nc.scalar.activation(lnv2r[:], psS2[0:1, :], Act.Ln,
                                         scale=1.0 / H, bias=eps_sb[0:1, :])
                    rstd2r = b2.tile([1, BPC], dt.float32)
                    nc.scalar.activation(rstd2r[:], lnv2r[:], Act.Exp,
                                         scale=-0.5)
                    # nh = hc * rstd (bcast via K=1 matmul) ... then *g+be
                    psb = b2ps.tile([H, BPC], dt.float32)
                    for sl in _chunks(BPC):
                        nc.tensor.matmul(psb[:, sl], onesrow[:, 0:H],
                                         rstd2r[:, sl], start=True, stop=True)
                    rsb = b2.tile([H, BPC], dt.float32)
                    nc.vector.tensor_copy(rsb[:], psb[:])
                    nh2 = b2.tile([H, BPC], dt.float32)
                    nc.vector.tensor_tensor(nh2[:], ps2c[:], rsb[:], Alu.mult)
                    g2 = np.broadcast_to(g21[:, None], (H, 1)).copy()
                    be2 = np.broadcast_to(be21[:, None], (H, 1)).copy()
                    c_g2 = nc.inline_tensor(g2.astype(np.float32), "c_g2")
                    c_be2 = nc.inline_tensor(be2.astype(np.float32), "c_be2")
                    g2_sb = b2.tile([H, 1], dt.float32)
                    nc.sync.dma_start(g2_sb[:], c_g2.ap())
                    be2_sb = b2.tile([H, 1], dt.float32)
                    nc.sync.dma_start(be2_sb[:], c_be2.ap())
                    nc.vector.tensor_scalar(nh2[:], nh2[:], g2_sb[:],
                                            be2_sb[:], Alu.mult, Alu.add)
                    nc.scalar.activation(rl2[:], nh2[:], Act.Relu)
                else:
                    nc.scalar.activation(rl2[:], ps2c[:], Act.Relu)
                psD = b2ps.tile([1, BPC], dt.float32)
                for sl in _chunks(BPC):
                    nc.tensor.matmul(psD[0:1, sl], w22_sb, rl2[:, sl],
                                     start=True, stop=True)
                # xd = sigmoid(rstd2*dots2 + b22)  (fast path)
                #      sigmoid(dots2 + b22)        (affine path: rstd inside)
                xdrow = b2.tile([1, BPC], dt.float32, tag="xdrow")
                if ln_affine:
                    en2a = b2.tile([1, BPC], dt.float32)
                    nc.scalar.activation(en2a[:], psD[0:1, :], Act.Exp,
                                         scale=-1.0, bias=nb22_sb[0:1, :])
                    nc.vector.tensor_scalar(en2a[:], en2a[:], 1.0, None,
                                            Alu.add)
                    nc.vector.reciprocal(xdrow[:], en2a[:])
                else:
                    lnv2 = b2.tile([1, BPC], dt.float32, tag="lnv2")
                    nc.scalar.activation(lnv2[:], psS2[0:1, :], Act.Ln,
                                         scale=1.0 / H, bias=eps_sb[0:1, :])
                    rstd2 = b2.tile([1, BPC], dt.float32, tag="rstd2")
                    nc.scalar.activation(rstd2[:], lnv2[:], Act.Exp,
                                         scale=-0.5)
                    xin2 = b2.tile([1, BPC], dt.float32, tag="xin2")
                    nc.vector.tensor_tensor(xin2[:], psD[0:1, :], rstd2[:],
                                            Alu.mult)
                    en2 = b2.tile([1, BPC], dt.float32, tag="en2")
                    nc.scalar.activation(en2[:], xin2[:], Act.Exp,
                                         scale=-1.0, bias=nb22_sb[0:1, :])
                    nc.vector.tensor_scalar(en2[:], en2[:], 1.0, None, Alu.add)
                    nc.vector.reciprocal(xdrow[:], en2[:])
                # broadcast xd over partitions: K=1 ones matmul
                psX = b2ps.tile([128, BPC], dt.float32)
                for sl in _chunks(BPC):
                    nc.tensor.matmul(psX[:, sl], onesrow[:], xdrow[:, sl],
                                     start=True, stop=True)
                nc.vector.tensor_copy(xdbc_sb[:], psX[:])

            # ================= main loop over groups of 8 b =================
            with tc.tile_pool(name="xt", bufs=6) as xpool, \
                 tc.tile_pool(name="psh", bufs=2, space="PSUM") as pshpool, \
                 tc.tile_pool(name="psx", bufs=5, space="PSUM") as psxpool, \
                 tc.tile_pool(name="big", bufs=2) as bigp, \
                 tc.tile_pool(name="sm", bufs=8) as smp:
                assert NG % 4 == 0
                for p_u in range(unroll * (NG // 4)):
                    p = p_u % (NG // 4)
                    # per-pair staging for 16-wide scalar chain
                    sqs = smp.tile([128, 32], dt.float32, tag="sqs")
                    dots = smp.tile([128, 32], dt.float32, tag="dots")
                    vx = smp.tile([128, 32], dt.float32, tag="vx")
                    nc.vector.tensor_scalar(vx[:],
                                            xdbc_sb[:, p * 32:(p + 1) * 32],
                                            w31, b3 + 1e-5, Alu.mult, Alu.add)
                    pss = []
                    sqv2 = bigp.tile([128, 2048], dt.float32, tag="sqv2")
                    rl2w = bigp.tile([128, 2048], dt.float32, tag="rl2w")
                    xt2w = bigp.tile([128, 2048], dt.float32, tag="xt2w")
                    # -------- phase 1: per-group heavy ops --------
                    for k in range(4):
                        g = 4 * p + k
                        xt = xpool.tile([T + 1, 8 * (N + 1)], dt.float32)
                        nc.sync.dma_start(xt[:],
                                          x_in[:, g * 8:(g + 1) * 8, :])
                        ps_h = pshpool.tile([128, 512], dt.float32)
                        ps_x = psxpool.tile([128, 512], dt.float32)
                        for j in range(8):
                            lhs = xt[:, j * (N + 1): j * (N + 1) + N]
                            nc.tensor.matmul(ps_h[:, j * H:(j + 1) * H],
                                             lhs, RW_sb[:, 0:H], start=True,
                                             stop=True)
                            nc.tensor.matmul(ps_x[:, j * T:(j + 1) * T],
                                             lhs, RW_sb[:, H:2 * H],
                                             start=True, stop=True)
                        hsl = slice(k * 512, (k + 1) * 512)
                        nc.scalar.copy(xt2w[:, hsl], ps_x[:])
                        nc.scalar.square(sqv2[:, hsl], ps_h[:])
                        if not ln_affine:
                            nc.scalar.activation(rl2w[:, hsl], ps_h[:],
                                                 Act.Relu)
                        pss.append(ps_h)

                    # -------- phase 2: pair-wide reductions + rstd --------
                    nc.vector.tensor_reduce(
                        sqs[:], sqv2[:].rearrange("p (g t) -> p g t", g=32),
                        Ax.X, Alu.add)
                    if not ln_affine:
                        dotp2 = bigp.tile([128, 2048], dt.float32, tag="dotp2")
                        w12v2 = w12_sb[:].unsqueeze(1).broadcast_to(
                            [128, 32, H])
                        nc.vector.tensor_tensor(
                            dotp2[:].rearrange("p (g t) -> p g t", g=32),
                            rl2w[:].rearrange("p (g t) -> p g t", g=32),
                            w12v2, Alu.mult)
                        nc.vector.tensor_reduce(
                            dots[:],
                            dotp2[:].rearrange("p (g t) -> p g t", g=32),
                            Ax.X, Alu.add)
                    lnv = smp.tile([128, 32], dt.float32, tag="lnv")
                    nc.scalar.activation(lnv[:], sqs[:], Act.Ln,
                                         scale=1.0 / H, bias=eps_sb[:])
                    rstd = smp.tile([128, 32], dt.float32, tag="rstd")
                    nc.scalar.activation(rstd[:], lnv[:], Act.Exp, scale=-0.5)
                    if ln_affine:
                        for k in range(4):
                            ps = pss[k]
                            nh = bigp.tile([128, 512], dt.float32, tag="nh")
                            nh3 = nh[:].rearrange("p (g t) -> p g t", g=8)
                            rst_b = rstd[:, k * 8:(k + 1) * 8].unsqueeze(
                                2).broadcast_to([128, 8, H])
                            nc.vector.tensor_tensor(
                                nh3,
                                ps[:].rearrange("p (g t) -> p g t", g=8),
                                rst_b, Alu.mult)
                            g_b = g11_sb[:].unsqueeze(1).broadcast_to(
                                [128, 8, H])
                            be_b = be11_sb[:].unsqueeze(1).broadcast_to(
                                [128, 8, H])
                            nc.vector.tensor_tensor(nh3, nh3, g_b, Alu.mult)
                            nc.vector.tensor_tensor(nh3, nh3, be_b, Alu.add)
                            rl = bigp.tile([128, 512], dt.float32, tag="rl")
                            nc.vector.tensor_scalar(rl[:], nh[:], 0.0, None,
                                                    Alu.max)
                            dotp = bigp.tile([128, 512], dt.float32,
                                             tag="dotp")
                            w12v = w12_sb[:].unsqueeze(1).broadcast_to(
                                [128, 8, H])
                            nc.gpsimd.tensor_tensor(
                                dotp[:].rearrange("p (g t) -> p g t", g=8),
                                rl[:].rearrange("p (g t) -> p g t", g=8),
                                w12v, Alu.mult)
                            nc.vector.tensor_reduce(
                                dots[:, k * 8:(k + 1) * 8],
                                dotp[:].rearrange("p (g t) -> p g t", g=8),
                                Ax.X, Alu.add)

                    # -------- phase 3: 16-wide scalar chain --------
                    xin = smp.tile([128, 32], dt.float32, tag="xin")
                    if ln_affine:
                        nc.vector.tensor_copy(xin[:], dots[:])
                    else:
                        nc.vector.tensor_tensor(xin[:], dots[:], rstd[:],
                                                Alu.mult)
                    exu = smp.tile([128, 32], dt.float32, tag="exu")
                    nc.scalar.activation(exu[:], xin[:], Act.Exp, scale=-1.0,
                                         bias=nb12_sb[:])
                    nc.vector.tensor_scalar(exu[:], exu[:], 1.0, None, Alu.add)
                    xu = smp.tile([128, 32], dt.float32, tag="xu")
                    nc.vector.reciprocal(xu[:], exu[:])
                    # v + 1e-5 = w30*xu + (w31*xd + b3 + 1e-5)
                    v1 = smp.tile([128, 32], dt.float32, tag="v1")
                    nc.vector.scalar_tensor_tensor(v1[:], xu[:], w30, vx[:],
                                                   Alu.mult, Alu.add)
                    rr = smp.tile([128, 32], dt.float32, tag="rr")
                    nc.vector.reciprocal(rr[:], v1[:])
                    fden = smp.tile([128, 32], dt.float32, tag="fden")
                    nc.vector.tensor_scalar(fden[:], rr[:], 50.0 * alpha, 1.0,
                                            Alu.mult, Alu.add)
                    F = smp.tile([128, 32], dt.float32, tag="F")
                    nc.vector.reciprocal(F[:], fden[:])
                    q = smp.tile([128, 32], dt.float32, tag="q")
                    nc.vector.tensor_scalar(q[:], F[:], -1.0, 1.0, Alu.mult,
                                            Alu.add)
                    if general_tail:
                        y5 = smp.tile([128, 32], dt.float32, tag="y5")
                        nc.vector.tensor_scalar(y5[:], rr[:], 5.0, 0.5,
                                                Alu.mult, Alu.add)
                        yi = smp.tile([128, 32], dt.int32, tag="yi")
                        nc.vector.tensor_copy(yi[:], y5[:])
                        yf = smp.tile([128, 32], dt.float32, tag="yf")
                        nc.vector.tensor_copy(yf[:], yi[:])
                        Tc = smp.tile([128, 32], dt.float32, tag="Tc")
                        nc.vector.tensor_scalar(Tc[:], yf[:], 0.0, 63.0,
                                                Alu.max, Alu.min)
                        mst = smp.tile([128, 32], dt.float32, tag="mst")
                        nc.vector.tensor_scalar(mst[:], Tc[:], -1.0, 63.0,
                                                Alu.mult, Alu.add)
                        men = smp.tile([128, 32], dt.float32, tag="men")
                        nc.vector.tensor_scalar(men[:], Tc[:], -1.0, 64.0,
                                                Alu.mult, Alu.add)

                    # ---- phase 4: pair-wide a-build + scan + accumulate ----
                    a2 = bigp.tile([128, 2048], dt.float32, tag="a2")
                    qb2 = q[:].rearrange("p (a g) -> p a g", a=4).unsqueeze(
                        3).broadcast_to([128, 4, 8, T])
                    t0v2 = t0m_sb[:].rearrange(
                        "p (g t) -> p g t", g=8).unsqueeze(1).broadcast_to(
                        [128, 4, 8, T])
                    nc.vector.tensor_tensor(
                        a2[:].rearrange("p (a g t) -> p a g t", a=4, g=8),
                        qb2, t0v2, Alu.mult)
                    s2 = bigp.tile([128, 2048], dt.float32, tag="s2")
                    nc.vector.tensor_tensor_scan(
                        s2[:], a2[:], xt2w[:], 0.0, Alu.mult, Alu.add)
                    accs = acc_sb[:, p * 32:(p + 1) * 32]
                    if general_tail:
                        sstar = smp.tile([128, 32], dt.float32, tag="sstar")
                        junk = bigp.tile([128, 64], dt.float32, tag="junk")
                        for j in range(16):
                            nc.vector.tensor_mask_reduce(
                                junk[:], s2[:, j * T:(j + 1) * T],
                                mst[:, j:j + 1], men[:, j:j + 1], 1.0,
                                -3.0e38, Alu.max,
                                accum_out=sstar[:, j:j + 1])
                        nc.vector.tensor_tensor(accs, sstar[:], F[:],
                                                Alu.mult)
                    else:
                        slast = s2[:].rearrange(
                            "p (g t) -> p g t", g=32)[:, :, T - 1]
                        nc.vector.tensor_tensor(accs, slast, F[:], Alu.mult)

                # ---- final: pred[b] = sum over partitions of acc ----
                with tc.tile_pool(name="fin", bufs=1) as fin, \
                     tc.tile_pool(name="finps", bufs=1, space="PSUM") as fps:
                    po = fps.tile([1, BPC], dt.float32)
                    for sl in _chunks(BPC):
                        nc.tensor.matmul(po[0:1, sl], ones_sb[:],
                                         acc_sb[:, sl], start=True, stop=True)
                    pred = fin.tile([1, BPC], dt.float32)
                    nc.vector.tensor_copy(pred[:], po[0:1, :])
                    nc.sync.dma_start(
                        y_out.rearrange("b one -> one b"), pred[:])

    # Force all activations onto the one table set that contains every
    # function we use (Relu/Square/Ln/Exp/Copy/Identity), so the compiled
    # stream has a single ACT table load instead of per-group thrash.
    # The pass picks the first listed set containing each function; ids must
    # stay aligned with act_info.json order, so empty out the other sets.
    import types
    from concourse.hw_specs import get_activation_tables
    import concourse._compat as _cc
    orig_tables = list(get_activation_tables(nc.m.arch).items())
    patched_tables = [
        (name, s if name == "natural_log_exp_and_others" else set())
        for name, s in orig_tables
    ]
    import bass_rust as _bass_rust_mod

    def _patched_act_loads(self):
        has_activation = any(
            type(i).__name__ == "InstActivation"
            for b in self.main_func.blocks
            for i in b.instructions
        )
        if not has_activation:
            return
        _bass_rust_mod.insert_act_table_loads(self, patched_tables)

    nc.insert_act_table_loads = types.MethodType(_patched_act_loads, nc)

    nc.compile()
    built = _Built()
    built.nc = nc
    built.BPC = BPC
    return built




def _build(weights, BPC, ln_affine, general_tail, unroll=1):
    """Poly fast path when valid, else the scan kernel."""
    if (not ln_affine) and (not general_tail) and poly_gate(weights):
        return build_poly(weights, BPC, unroll=unroll)
    return _build_scan(weights, BPC, ln_affine, general_tail, unroll=unroll)


_CACHE = {}


def _get_built(weights, BPC, ln_affine, general_tail):
    full_key = (BPC, ln_affine, general_tail,
                b"".join(_np32(weights[k]).tobytes() for k in sorted(weights)))
    if full_key not in _CACHE:
        _CACHE[full_key] = _build(weights, BPC, ln_affine, general_tail)
    return _CACHE[full_key]


def _fold_weights(inputs):
    mean = float(np.asarray(inputs["x_mean"]))
    std = float(np.asarray(inputs["x_std"]))
    W11r = _np32(inputs["W11"])
    W21r = _np32(inputs["W21"])
    w = {
        "W11": W11r / std,
        "b11": _np32(inputs["b11"]) - (mean / std) * W11r.sum(0),
        "W21": W21r / std,
        "b21": _np32(inputs["b21"]) - (mean / std) * W21r.sum(0),
        "W12": _np32(inputs["W12"])[:, 0],
        "b12": float(np.asarray(inputs["b12"])[0]),
        "W22": _np32(inputs["W22"])[:, 0],
        "b22": float(np.asarray(inputs["b22"])[0]),
        "g11": _np32(inputs["g11"]), "be11": _np32(inputs["be11"]),
        "g21": _np32(inputs["g21"]), "be21": _np32(inputs["be21"]),
        "w30": float(np.asarray(inputs["W3"])[0, 0]),
        "w31": float(np.asarray(inputs["W3"])[1, 0]),
        "b3": float(np.asarray(inputs["b3"])[0]),
        "alpha": float(np.asarray(inputs["alpha"])[0]),
    }
    return w


def _tail_is_degenerate(w):
    """True iff v+1e-5 is provably inside (-10+m, -m) for all sigmoid outputs,
    which forces round(Tv/10) <= -1 -> T_idx clamps to 0 -> Ln == 64."""
    lo = w["b3"] + 1e-5 + min(w["w30"], 0.0) + min(w["w31"], 0.0)
    hi = w["b3"] + 1e-5 + max(w["w30"], 0.0) + max(w["w31"], 0.0)
    m = 1e-3
    return (lo > -10.0 + m) and (hi < -m) and w["alpha"] >= 0.0


def _use_poly(w, ln_affine, general_tail):
    return (not ln_affine) and (not general_tail) and poly_gate(w)


def make_in_maps(x, poly):
    """Per-core input staging. poly: bf16 t-major 2D x + f32 down channel.
    scan fallback: f32 t-major padded x + f32 down channel."""
    BPC = B // N_CORES
    in_maps = []
    bf = mybir.dt.np(dt.bfloat16)
    for c in range(N_CORES):
        xs = x[c * BPC:(c + 1) * BPC]          # [BPC, T, N+1]
        xp = np.empty((T + 1, BPC, N + 1), np.float32)
        xp[:T] = xs.transpose(1, 0, 2)
        xp[T] = 1.0
        d = np.ascontiguousarray(xp[:, :, N])
        if poly:
            in_maps.append({"x": np.ascontiguousarray(
                xp.astype(bf).reshape(T + 1, BPC * (N + 1))),
                "d": d.astype(bf)})
        else:
            in_maps.append({"x": xp, "d": d})
    return in_maps


def kernel(**inputs) -> np.ndarray:
    x = _np32(inputs["x"])
    assert x.shape == (B, T, N + 1)
    w = _fold_weights(inputs)
    ln_affine = not (np.all(w["g11"] == 1.0) and np.all(w["be11"] == 0.0)
                     and np.all(w["g21"] == 1.0) and np.all(w["be21"] == 0.0))
    general_tail = not _tail_is_degenerate(w)
    BPC = B // N_CORES
    built = _get_built(w, BPC, ln_affine, general_tail)
    in_maps = make_in_maps(x, _use_poly(w, ln_affine, general_tail))
    res = run_bass_kernel_spmd(built.nc, in_maps, list(range(N_CORES)))
    out = np.concatenate([r["y"] for r in res.results], axis=0)
    return out.astype(np.float32)


if __name__ == "__main__":
    print("kernel module ok")



# revision 7
# speedup vs baseline: 1.3931x; 1.3931x over previous
"""Trainium2 Bass kernel for nn_Diffusion_Model (ragged_sequence).

Pure data-parallel: batch B=4096 sharded as 512 per NeuronCore across 8 cores.

Fast path (staged weights): the ragged tail is degenerate (Ln == 64 for all
(b, n)) and |q-1| <= 0.0055, so the geometric tail sum_t q^(63-t) u_t is a
degree-4 polynomial in delta = q-1 whose coefficient sums G_k = sum_t
C(63-t,k) u_t come out of the PE as 5 extra (static-weight) matmul columns.
The relu dot uses sum W12 relu(h) = (d0 + sum W12|h|)/2 with d0 as one more
matmul column. No transpose, no scan, no per-token masking. bf16 input DMA +
bf16 matmuls; Pool does the squares and |h|*W12 products; DVE only pair-adds
and the two segmented reduces; per-(b,n) scalar chain runs batch-wide once.

Fallback (general weights): the previous scan-based kernel (PE transpose +
DVE Horner scan, tensor_mask_reduce extraction for ragged Ln).
"""
import sys
import numpy as np

sys.path.insert(0, "/opt/trn_rl_repo")

import concourse.bacc as bacc
import concourse.tile as tile
import concourse.mybir as mybir
from concourse.bass_utils import run_bass_kernel_spmd

dt = mybir.dt
Alu = mybir.AluOpType
Act = mybir.ActivationFunctionType
Ax = mybir.AxisListType

N_CORES = 8
B, T, N, H = 4096, 64, 128, 64
LN_EPS = 1e-5


def _np32(a):
    return np.ascontiguousarray(np.asarray(a, dtype=np.float32))


class _Built:
    pass


NPOLY = 5          # delta powers 0..4
GB = 16            # batches per group
NCOL = NPOLY + 1   # d0 column + NPOLY G columns


def poly_gate(w):
    """True iff the polynomial tail is valid: Ln==64 everywhere and
    63*|q-1| small enough for a degree-4 expansion."""
    lo = w["b3"] + 1e-5 + min(w["w30"], 0.0) + min(w["w31"], 0.0)
    hi = w["b3"] + 1e-5 + max(w["w30"], 0.0) + max(w["w31"], 0.0)
    m = 1e-3
    degenerate = (lo > -10.0 + m) and (hi < -m) and w["alpha"] >= 0.0
    if not degenerate:
        return False
    a50 = 50.0 * w["alpha"]
    d_lo, d_hi = a50 + lo, a50 + hi
    if d_lo <= 1.0:
        return False
    dmax = max(abs(a50 / d_lo - 1.0), abs(a50 / d_hi - 1.0))
    return 63.0 * dmax <= 0.5


def _bf16(a):
    return np.asarray(a, dtype=np.float32).astype(mybir.dt.np(dt.bfloat16))


# Per-pair engine assignment for the elementwise stages (16 pairs of 2
# groups).  'D' = DVE, 'P' = gpsimd/Pool, 'A' = ACT (sq only).
SQ_ENG = ['A' if p % 3 == 2 else 'D' for p in range(16)]
MDW_ENG = ['D'] * 16
PA_ENG = ['P' if p % 3 == 1 else 'D' for p in range(16)]
PB_ENG = ['P' if p % 2 == 0 else 'D' for p in range(16)]
PC_ENG = ['P' if p % 2 == 1 else 'D' for p in range(16)]
PD_ENG = ['P' if p % 2 == 0 else 'D' for p in range(16)]
HORNER_ENG = ['P', 'D', 'D']


def build_poly(weights, BPC, unroll=1, hwloop=None, variant="full"):
    NG = BPC // GB                    # 32 groups
    NPAIR = NG // 2                   # 16 pairs
    CHUNK_GROUPS = [12, 12, 8]        # tail chunks (groups per chunk)
    assert sum(CHUNK_GROUPS) == NG
    nc = bacc.Bacc("TRN2", target_bir_lowering=False, debug=False,
                   num_devices=N_CORES)

    x_in = nc.dram_tensor("x", [T + 1, BPC * (N + 1)], dt.bfloat16,
                          kind="ExternalInput").ap()
    d_in = nc.dram_tensor("d", [T + 1, BPC], dt.bfloat16,
                          kind="ExternalInput").ap()
    y_out = nc.dram_tensor("y", [BPC, 1], dt.float32,
                           kind="ExternalOutput").ap()

    W11, b11 = weights["W11"], weights["b11"]
    W21, b21 = weights["W21"], weights["b21"]
    W12, b12 = weights["W12"], float(weights["b12"])
    W22, b22 = weights["W22"], float(weights["b22"])
    w30, w31, b3 = (float(weights["w30"]), float(weights["w31"]),
                    float(weights["b3"]))
    alpha = float(weights["alpha"])
    a50 = 50.0 * alpha

    # LN mean-centering folded into the weights (exact linear algebra).
    W11c = (W11.astype(np.float64)
            - W11.astype(np.float64).mean(1, keepdims=True)).astype(np.float32)
    b11c = (b11.astype(np.float64) - b11.astype(np.float64).mean()).astype(np.float32)
    W21c = (W21.astype(np.float64)
            - W21.astype(np.float64).mean(1, keepdims=True)).astype(np.float32)
    b21c = (b21.astype(np.float64) - b21.astype(np.float64).mean()).astype(np.float32)

    RWm = np.zeros((T + 1, H), np.float32)
    RWm[:T] = W11c
    RWm[T] = b11c
    # G columns: col 0 = d0 (W11c @ W12/2, relu linear part); cols 1..5 =
    # NEGATED binomial tail weights C(63-t, k) so that accm = m*s equals
    # +F*poly and pred is a plain partition-sum (gpsimd C-reduce).
    RWG = np.zeros((T + 1, NCOL), np.float32)
    w12h = 0.5 * W12.astype(np.float64)
    RWG[:T, 0] = (W11c.astype(np.float64) @ w12h).astype(np.float32)
    RWG[T, 0] = float(b11c.astype(np.float64) @ w12h)
    from math import comb
    for k in range(NPOLY):
        RWG[:T, k + 1] = [-float(comb(63 - int(t), k)) for t in range(T)]

    RW2 = np.zeros((T + 1, H), np.float32)
    RW2[:T] = W21c
    RW2[T] = b21c

    w12bc = np.broadcast_to((0.5 * W12)[None, :], (128, H)).copy()
    w22_col = W22.reshape(T, 1).astype(np.float32)

    # pack all [65-row] bf16 constants into one tensor -> one DMA
    CPK = np.zeros((T + 1, H + NCOL + H + 2), np.float32)
    CPK[:, 0:H] = RWm
    CPK[:, H:H + NCOL] = RWG
    CPK[:, H + NCOL:2 * H + NCOL] = RW2
    CPK[:T, 2 * H + NCOL] = 1.0
    CPK[:T, 2 * H + NCOL + 1] = w22_col[:, 0]
    c_CPK = nc.inline_tensor(_bf16(CPK), "c_CPK")
    c_w12bc = nc.inline_tensor(_bf16(w12bc), "c_w12bc")

    def _chunks(total, step=512):
        return [slice(i, min(i + step, total)) for i in range(0, total, step)]

    with tile.TileContext(nc) as tc:
        with tc.tile_pool(name="const", bufs=1) as cpool:
            CPK_sb = cpool.tile([T + 1, H + NCOL + H + 2], dt.bfloat16)
            nc.sync.dma_start(CPK_sb[:], c_CPK.ap())
            RWm_sb = CPK_sb[:, 0:H]
            RWG_sb = CPK_sb[:, H:H + NCOL]
            RW2_sb = CPK_sb[:, H + NCOL:2 * H + NCOL]
            ones64_sb = CPK_sb[0:T, 2 * H + NCOL:2 * H + NCOL + 1]
            w22_sb = CPK_sb[0:T, 2 * H + NCOL + 1:2 * H + NCOL + 2]
            w12_sb = cpool.tile([128, H], dt.bfloat16)
            nc.sync.dma_start(w12_sb[:], c_w12bc.ap())

            eps_sb = cpool.tile([128, 1], dt.float32)
            nc.vector.memset(eps_sb[:], LN_EPS)
            one_sb = cpool.tile([128, 1], dt.float32)
            nc.vector.memset(one_sb[:], 1.0)
            nb12_sb = cpool.tile([128, 1], dt.float32)
            nc.vector.memset(nb12_sb[:], -b12)
            nb22_sb = cpool.tile([128, 1], dt.float32)
            nc.vector.memset(nb22_sb[:], -b22)
            onesrow_sb = cpool.tile([1, 128], dt.float32)
            nc.vector.memset(onesrow_sb[:], 1.0)
            posones_sb = cpool.tile([128, 1], dt.bfloat16)
            nc.vector.memset(posones_sb[:], 1.0)

            with tc.tile_pool(name="acc", bufs=2) as apool, \
                 tc.tile_pool(name="b2", bufs=2) as b2, \
                 tc.tile_pool(name="b2ps", bufs=1, space="PSUM") as b2ps, \
                 tc.tile_pool(name="xt", bufs=6) as xpool, \
                 tc.tile_pool(name="psh", bufs=2, space="PSUM") as pshp, \
                 tc.tile_pool(name="psg", bufs=2, space="PSUM") as psgp, \
                 tc.tile_pool(name="habsp", bufs=3) as habsp, \
                 tc.tile_pool(name="mdp", bufs=3) as mdp, \
                 tc.tile_pool(name="pap", bufs=3) as papool, \
                 tc.tile_pool(name="pbp", bufs=3) as pbpool, \
                 tc.tile_pool(name="tail", bufs=2) as tp, \
                 tc.tile_pool(name="tailps", bufs=1, space="PSUM") as tps:
                import contextlib
                loop_ctxs = []
                if hwloop is not None and hwloop > 1:
                    unroll = 1
                for rep in range(unroll):
                    ctx = (tc.For_i(0, hwloop, 1,
                                    hint_engines=(mybir.EngineType.PE,),
                                    name="reploop")
                           if hwloop is not None and hwloop > 1
                           else contextlib.nullcontext())
                    loop_ctxs.append(ctx)
                    ctx.__enter__()
                    CB_sb = apool.tile([128, BPC], dt.float32, tag="CB")
                    pred = apool.tile([1, BPC], dt.float32, tag="pred")

                    do_b2 = variant != "dma_only"
                    do_compute = variant != "dma_only"
                    # ---- branch 2 (down channel) -> CB = w31*xd + const,
                    # broadcast over partitions; overlaps the group loop ----
                    dstage = b2.tile([T + 1, BPC], dt.bfloat16, tag="dstage")
                    nc.sync.dma_start(dstage[:], d_in[:])
                    if do_b2:
                        ps2c = b2ps.tile([128, BPC], dt.float32, tag="big")
                        for sl in _chunks(BPC):
                            nc.tensor.matmul(ps2c[0:H, sl], RW2_sb,
                                             dstage[:, sl], start=True, stop=True)
                        sq2 = b2.tile([H, BPC], dt.bfloat16, tag="sq2")
                        nc.scalar.square(sq2[:], ps2c[0:H, :])
                        psS2 = b2ps.tile([128, BPC], dt.float32, tag="big")
                        for sl in _chunks(BPC):
                            nc.tensor.matmul(psS2[0:1, sl], ones64_sb,
                                             sq2[:, sl], start=True, stop=True)
                        rl2 = b2.tile([H, BPC], dt.bfloat16, tag="rl2")
                        nc.scalar.activation(rl2[:], ps2c[0:H, :], Act.Relu)
                        lnv2 = b2.tile([1, BPC], dt.float32, tag="lnv2")
                        nc.scalar.activation(lnv2[:], psS2[0:1, :], Act.Ln,
                                             scale=1.0 / H, bias=eps_sb[0:1, :])
                        rstd2 = b2.tile([1, BPC], dt.float32, tag="rstd2")
                        nc.scalar.activation(rstd2[:], lnv2[:], Act.Exp,
                                             scale=-0.5)
                        psD = b2ps.tile([128, BPC], dt.float32, tag="big")
                        for sl in _chunks(BPC):
                            nc.tensor.matmul(psD[0:1, sl], w22_sb, rl2[:, sl],
                                             start=True, stop=True)
                        xin2 = b2.tile([1, BPC], dt.float32, tag="xin2")
                        nc.vector.tensor_tensor(xin2[:], psD[0:1, :], rstd2[:],
                                                Alu.mult)
                        en2 = b2.tile([1, BPC], dt.float32, tag="en2")
                        nc.scalar.activation(en2[:], xin2[:], Act.Exp,
                                             scale=-1.0, bias=nb22_sb[0:1, :])
                        nc.vector.tensor_scalar(en2[:], en2[:], 1.0, None,
                                                Alu.add)
                        xdrow = b2.tile([1, BPC], dt.float32, tag="xdrow")
                        nc.vector.reciprocal(xdrow[:], en2[:])
                        # fold CB = w31*xd + (b3 + 1e-5 + 50*alpha) pre-broadcast
                        crow = b2.tile([1, BPC], dt.float32, tag="crow")
                        nc.gpsimd.tensor_scalar(crow[:], xdrow[:], w31,
                                                b3 + 1e-5 + a50, Alu.mult,
                                                Alu.add)
                        psC = b2ps.tile([128, BPC], dt.float32, tag="big")
                        for sl in _chunks(BPC):
                            nc.tensor.matmul(psC[:, sl], onesrow_sb[:],
                                             crow[:, sl], start=True, stop=True)
                        nc.scalar.copy(CB_sb[:], psC[:])

                    # ---- main loop: chunks x pairs of groups of GB ----
                    w12v = w12_sb[:].unsqueeze(1).broadcast_to([128, 2 * GB, H])
                    if variant == "nodma":
                        xt_static = xpool.tile([T + 1, 2 * GB * (N + 1)],
                                               dt.bfloat16, tag="xts")
                        nc.sync.dma_start(
                            xt_static[:], x_in[:, 0:2 * GB * (N + 1)])
                    g0 = 0
                    for c, GPC in enumerate(CHUNK_GROUPS):
                        CW = GPC * GB
                        comb_c = tp.tile([128, 2 * CW], dt.float32,
                                         tag=f"comb{c}")
                        G_c = tp.tile([128, CW * NCOL], dt.bfloat16,
                                      tag=f"G{c}")
                        for pp in range(GPC // 2):
                            pg = (g0 + 2 * pp) // 2   # global pair idx
                            habs2 = habsp.tile([128, 2 * GB * H], dt.bfloat16,
                                               tag="habs2")
                            for half in range(2):
                                g = g0 + 2 * pp + half
                                if variant == "nodma":
                                    xt = xt_static
                                elif g % 2 == 0:
                                    xt = xpool.tile(
                                        [T + 1, 2 * GB * (N + 1)],
                                        dt.bfloat16)
                                    nc.sync.dma_start(
                                        xt[:],
                                        x_in[:, g * GB * (N + 1):
                                             (g + 2) * GB * (N + 1)])
                                xoff = (g % 2) * GB * (N + 1)
                                if not do_compute:
                                    continue
                                ps_h = pshp.tile([128, GB * H], dt.float32)
                                if half == 0:
                                    ps_g2 = psgp.tile(
                                        [128, 2 * GB * NCOL], dt.float32)
                                ps_g = ps_g2[:, half * GB * NCOL:
                                             (half + 1) * GB * NCOL]
                                for j in range(GB):
                                    lhs = xt[:, xoff + j * (N + 1):
                                             xoff + j * (N + 1) + N]
                                    nc.tensor.matmul(
                                        ps_h[:, j * H:(j + 1) * H], lhs,
                                        RWm_sb, start=True, stop=True)
                                    nc.tensor.matmul(
                                        ps_g[:, j * NCOL:(j + 1) * NCOL], lhs,
                                        RWG_sb, start=True, stop=True)
                                nc.scalar.activation(
                                    habs2[:, half * GB * H:(half + 1) * GB * H],
                                    ps_h[:], Act.Abs)
                            if do_compute:
                                gg0 = 2 * pp
                                nc.scalar.copy(
                                    G_c[:, gg0 * GB * NCOL:
                                        (gg0 + 2) * GB * NCOL],
                                    ps_g2[:])
                            if not do_compute:
                                continue
                            # ---- paired elementwise stages ----
                            PW = 2 * GB * H          # 2048
                            md = mdp.tile([128, 2 * PW], dt.bfloat16,
                                          tag="md")
                            sq_e = SQ_ENG[pg]
                            if sq_e == 'A':
                                nc.scalar.activation(md[:, 0:PW], habs2[:],
                                                     Act.Square)
                            else:
                                eng = nc.vector if sq_e == 'D' else nc.gpsimd
                                eng.tensor_tensor(md[:, 0:PW], habs2[:],
                                                  habs2[:], Alu.mult)
                            eng = nc.vector if MDW_ENG[pg] == 'D' else nc.gpsimd
                            eng.tensor_tensor(
                                md[:, PW:2 * PW].rearrange(
                                    "p (g t) -> p g t", g=2 * GB),
                                habs2[:].rearrange("p (g t) -> p g t",
                                                   g=2 * GB),
                                w12v, Alu.mult)
                            md3 = md[:].rearrange(
                                "p (two b t) -> p two b t", two=2, b=2 * GB)
                            pa = papool.tile([128, PW], dt.bfloat16, tag="pa")
                            pa3 = pa[:].rearrange(
                                "p (two b t) -> p two b t", two=2, b=2 * GB)
                            eng = nc.vector if PA_ENG[pg] == 'D' else nc.gpsimd
                            eng.tensor_tensor(pa3, md3[:, :, :, 0:H // 2],
                                              md3[:, :, :, H // 2:H], Alu.add)
                            pb = pbpool.tile([128, PW // 2], dt.bfloat16,
                                             tag="pb")
                            pb3 = pb[:].rearrange(
                                "p (two b t) -> p two b t", two=2, b=2 * GB)
                            eng = nc.vector if PB_ENG[pg] == 'D' else nc.gpsimd
                            eng.tensor_tensor(pb3, pa3[:, :, :, 0:H // 4],
                                              pa3[:, :, :, H // 4:H // 2],
                                              Alu.add)
                            pc = pbpool.tile([128, PW // 4], dt.bfloat16,
                                             tag="pc")
                            pc3 = pc[:].rearrange(
                                "p (two b t) -> p two b t", two=2, b=2 * GB)
                            eng = nc.vector if PC_ENG[pg] == 'D' else nc.gpsimd
                            eng.tensor_tensor(pc3, pb3[:, :, :, 0:H // 8],
                                              pb3[:, :, :, H // 8:H // 4],
                                              Alu.add)
                            pd = pbpool.tile([128, PW // 8], dt.bfloat16,
                                             tag="pd")
                            pd3 = pd[:].rearrange(
                                "p (two b t) -> p two b t", two=2, b=2 * GB)
                            eng = nc.vector if PD_ENG[pg] == 'D' else nc.gpsimd
                            eng.tensor_tensor(pd3, pc3[:, :, :, 0:H // 16],
                                              pc3[:, :, :, H // 16:H // 8],
                                              Alu.add)
                            poff = 2 * pp * GB
                            nc.vector.tensor_reduce(
                                comb_c[:].rearrange(
                                    "p (two b) -> p two b",
                                    two=2)[:, :, poff:poff + 2 * GB],
                                pd3, Ax.X, Alu.add)

                        # ---- tail for this chunk of CW batches ----
                        csl = slice(g0 * GB, g0 * GB + CW)
                        g0 += GPC
                        if not do_compute:
                            continue
                        Gv = G_c[:].rearrange("p (b k) -> p b k", k=NCOL)
                        sqv = comb_c[:, 0:CW]
                        d1v = comb_c[:, CW:2 * CW]
                        lnv = tp.tile([128, CW], dt.float32, tag="lnv")
                        nc.scalar.activation(lnv[:], sqv, Act.Ln,
                                             scale=1.0 / H, bias=eps_sb[:])
                        rstd = tp.tile([128, CW], dt.float32, tag="rstd")
                        nc.scalar.activation(rstd[:], lnv[:], Act.Exp,
                                             scale=-0.5)
                        dot = tp.tile([128, CW], dt.float32, tag="dot")
                        nc.vector.tensor_tensor(dot[:], d1v, Gv[:, :, 0],
                                                Alu.add)
                        xin = tp.tile([128, CW], dt.float32, tag="xin")
                        nc.vector.tensor_tensor(xin[:], dot[:], rstd[:],
                                                Alu.mult)
                        e = tp.tile([128, CW], dt.float32, tag="e")
                        nc.scalar.activation(e[:], xin[:], Act.Exp,
                                             scale=-1.0, bias=nb12_sb[:])
                        l1 = tp.tile([128, CW], dt.float32, tag="l1")
                        nc.scalar.activation(l1[:], e[:], Act.Ln, scale=1.0,
                                             bias=one_sb[:])
                        xu = tp.tile([128, CW], dt.float32, tag="xu")
                        nc.scalar.activation(xu[:], l1[:], Act.Exp,
                                             scale=-1.0)
                        D = tp.tile([128, CW], dt.float32, tag="D")
                        nc.vector.scalar_tensor_tensor(D[:], xu[:], w30,
                                                       CB_sb[:, csl],
                                                       Alu.mult, Alu.add)
                        lD = tp.tile([128, CW], dt.float32, tag="lD")
                        nc.scalar.activation(lD[:], D[:], Act.Ln,
                                             scale=1.0 / a50)
                        q = tp.tile([128, CW], dt.float32, tag="q")
                        nc.scalar.activation(q[:], lD[:], Act.Exp, scale=-1.0)
                        m = tp.tile([128, CW], dt.bfloat16, tag="m")
                        nc.vector.tensor_scalar(m[:], q[:], 1.0, None,
                                                Alu.subtract)
                        # Horner: s = G1 + m*(G2 + m*(G3 + m*(G4 + m*G5)))
                        heng = (nc.vector if HORNER_ENG[c % len(HORNER_ENG)]
                                == 'D' else nc.gpsimd)
                        t_ = tp.tile([128, CW], dt.bfloat16, tag="hm")
                        heng.tensor_tensor(t_[:], m[:], Gv[:, :, NPOLY],
                                           Alu.mult)
                        s = tp.tile([128, CW], dt.bfloat16, tag="hs")
                        heng.tensor_tensor(s[:], Gv[:, :, NPOLY - 1],
                                           t_[:], Alu.add)
                        for k in range(NPOLY - 2, 0, -1):
                            t_ = tp.tile([128, CW], dt.bfloat16, tag=f"hm{k}")
                            heng.tensor_tensor(t_[:], m[:], s[:], Alu.mult)
                            s = tp.tile([128, CW], dt.bfloat16, tag=f"hs{k}")
                            heng.tensor_tensor(s[:], Gv[:, :, k], t_[:],
                                               Alu.add)
                        accm = tp.tile([128, CW], dt.bfloat16, tag="accm")
                        heng.tensor_tensor(accm[:], m[:], s[:], Alu.mult)
                        # pred[b] = sum_n accm  (G negated: accm = +F*s)
                        po = tps.tile([1, BPC], dt.float32, tag="po")
                        nc.tensor.matmul(po[0:1, csl], posones_sb[:],
                                         accm[:], start=True, stop=True)
                        nc.scalar.copy(pred[0:1, csl], po[0:1, csl])

                    if not do_compute:
                        nc.vector.memset(pred[:], 0.0)
                    nc.sync.dma_start(
                        y_out.rearrange("b one -> one b"), pred[:])
                    loop_ctxs.pop().__exit__(None, None, None)

    # Pin every activation to the one table containing ln/exp/abs/relu/square
    # so the stream has a single ACT table load.
    import types
    from concourse.hw_specs import get_activation_tables
    orig_tables = list(get_activation_tables(nc.m.arch).items())
    patched_tables = [
        (name, s if name == "natural_log_exp_and_others" else set())
        for name, s in orig_tables
    ]
    import bass_rust as _bass_rust_mod

    def _patched_act_loads(self):
        has_activation = any(
            type(i).__name__ == "InstActivation"
            for b in self.main_func.blocks
            for i in b.instructions
        )
        if not has_activation:
            return
        _bass_rust_mod.insert_act_table_loads(self, patched_tables)

    nc.insert_act_table_loads = types.MethodType(_patched_act_loads, nc)

    nc.compile()

    built = _Built()
    built.nc = nc
    built.BPC = BPC
    return built


def _build_scan(weights, BPC, ln_affine, general_tail, unroll=1):
    """Build the per-core Bass program. weights: dict of host-folded arrays."""
    NG = BPC // 8  # groups of 8 batches
    nc = bacc.Bacc("TRN2", target_bir_lowering=False, debug=False,
                   num_devices=N_CORES)

    x_in = nc.dram_tensor("x", [T + 1, BPC, N + 1], dt.float32,
                          kind="ExternalInput").ap()
    d_in = nc.dram_tensor("d", [T + 1, BPC], dt.float32,
                          kind="ExternalInput").ap()
    y_out = nc.dram_tensor("y", [BPC, 1], dt.float32,
                           kind="ExternalOutput").ap()

    # ---- inline constants ----
    W11, b11 = weights["W11"], weights["b11"]     # [64,64],[64]
    W21, b21 = weights["W21"], weights["b21"]
    W12, b12 = weights["W12"], float(weights["b12"])   # [64], scalar
    W22, b22 = weights["W22"], float(weights["b22"])
    g11, be11 = weights["g11"], weights["be11"]
    g21, be21 = weights["g21"], weights["be21"]
    w30, w31, b3 = (float(weights["w30"]), float(weights["w31"]),
                    float(weights["b3"]))
    alpha = float(weights["alpha"])

    # fold LN mean-centering into the weights (exact linear algebra):
    # mean_j of (x@W + b) = x@rowmean(W) + mean(b); subtracting it is the
    # same matmul with row-centered W and mean-centered b.
    W11c = (W11.astype(np.float64)
            - W11.astype(np.float64).mean(1, keepdims=True)).astype(np.float32)
    b11c = (b11.astype(np.float64) - b11.astype(np.float64).mean()).astype(np.float32)
    W21c = (W21.astype(np.float64)
            - W21.astype(np.float64).mean(1, keepdims=True)).astype(np.float32)
    b21c = (b21.astype(np.float64) - b21.astype(np.float64).mean()).astype(np.float32)
    RW = np.zeros((T + 1, 2 * H), np.float32)
    RW[:T, :H] = W11c
    RW[T, :H] = b11c
    RW[:T, H:] = np.eye(T, dtype=np.float32)
    RW2 = np.zeros((T + 1, H), np.float32)
    RW2[:T, :] = W21c
    RW2[T, :] = b21c

    w12bc = np.broadcast_to(W12[None, :], (128, H)).copy()
    g11bc = np.broadcast_to(g11[None, :], (128, H)).copy()
    be11bc = np.broadcast_to(be11[None, :], (128, H)).copy()
    t0m = np.ones((128, 512), np.float32)
    t0m[:, ::T] = 0.0
    ones_col = np.ones((128, 1), np.float32)
    ones64_col = np.ones((T, 1), np.float32)
    noneg64_row = np.full((1, H), -1.0 / H, np.float32)
    w22_col = W22.reshape(T, 1).astype(np.float32)

    c_RW = nc.inline_tensor(RW, "c_RW")
    c_RW2 = nc.inline_tensor(RW2, "c_RW2")
    c_w12bc = nc.inline_tensor(w12bc, "c_w12bc")
    c_t0m = nc.inline_tensor(t0m, "c_t0m")
    c_ones = nc.inline_tensor(ones_col, "c_ones")
    c_ones64 = nc.inline_tensor(ones64_col, "c_ones64")
    c_negmean = nc.inline_tensor(noneg64_row, "c_negmean")
    c_w22 = nc.inline_tensor(w22_col, "c_w22")
    if ln_affine:
        c_g11bc = nc.inline_tensor(g11bc, "c_g11bc")
        c_be11bc = nc.inline_tensor(be11bc, "c_be11bc")

    def _chunks(total, step=512):
        return [slice(i, min(i + step, total)) for i in range(0, total, step)]

    with tile.TileContext(nc) as tc:
        with tc.tile_pool(name="const", bufs=1) as cpool:
            RW_sb = cpool.tile([T + 1, 2 * H], dt.float32)
            nc.sync.dma_start(RW_sb[:], c_RW.ap())
            RW2_sb = cpool.tile([T + 1, H], dt.float32)
            nc.sync.dma_start(RW2_sb[:], c_RW2.ap())
            w12_sb = cpool.tile([128, H], dt.float32)
            nc.sync.dma_start(w12_sb[:], c_w12bc.ap())
            t0m_sb = cpool.tile([128, 512], dt.float32)
            nc.sync.dma_start(t0m_sb[:], c_t0m.ap())
            ones_sb = cpool.tile([128, 1], dt.float32)
            nc.sync.dma_start(ones_sb[:], c_ones.ap())
            ones64_sb = cpool.tile([T, 1], dt.float32)
            nc.sync.dma_start(ones64_sb[:], c_ones64.ap())
            negmean_sb = cpool.tile([1, H], dt.float32)
            nc.sync.dma_start(negmean_sb[:], c_negmean.ap())
            w22_sb = cpool.tile([T, 1], dt.float32)
            nc.sync.dma_start(w22_sb[:], c_w22.ap())
            if ln_affine:
                g11_sb = cpool.tile([128, H], dt.float32)
                nc.sync.dma_start(g11_sb[:], c_g11bc.ap())
                be11_sb = cpool.tile([128, H], dt.float32)
                nc.sync.dma_start(be11_sb[:], c_be11bc.ap())

            eps_sb = cpool.tile([128, 1], dt.float32)
            nc.vector.memset(eps_sb[:], LN_EPS)
            b22_sb = cpool.tile([128, 1], dt.float32)
            nc.vector.memset(b22_sb[:], b22)
            nb12_sb = cpool.tile([128, 1], dt.float32)
            nc.vector.memset(nb12_sb[:], -b12)
            nb22_sb = cpool.tile([128, 1], dt.float32)
            nc.vector.memset(nb22_sb[:], -b22)

            xdbc_sb = cpool.tile([128, BPC], dt.float32)   # xd broadcast
            acc_sb = cpool.tile([128, BPC], dt.float32)    # F * s* per token

            # ================= branch 2 (down channel), once =================
            with tc.tile_pool(name="b2", bufs=1) as b2, \
                 tc.tile_pool(name="b2ps", bufs=1, space="PSUM") as b2ps:
                onesrow = b2.tile([1, 128], dt.float32)
                nc.vector.memset(onesrow[:], 1.0)
                dstage = b2.tile([T + 1, BPC], dt.float32)
                nc.sync.dma_start(dstage[:], d_in[:])
                ps2c = b2ps.tile([H, BPC], dt.float32)
                for sl in _chunks(BPC):
                    nc.tensor.matmul(ps2c[:, sl], RW2_sb[:], dstage[:, sl],
                                     start=True, stop=True)
                sq2 = b2.tile([H, BPC], dt.float32)
                nc.scalar.square(sq2[:], ps2c[:])
                psS2 = b2ps.tile([1, BPC], dt.float32)
                for sl in _chunks(BPC):
                    nc.tensor.matmul(psS2[0:1, sl], ones64_sb, sq2[:, sl],
                                     start=True, stop=True)
                rl2 = b2.tile([H, BPC], dt.float32)
                if ln_affine:
                    # general gamma/beta for branch 2: nh*g+be then relu
                    lnv2r = b2.tile([1, BPC], dt.float32)
                    nc.scalar.activation(lnv2r[:], psS2[0:1, :], Act.Ln,
                                         scale=1.0 / H, bias=eps_sb[0:1, :])
                    rstd2r = b2.tile([1, BPC], dt.float32)
                    nc.scalar.activation(rstd2r[:], lnv2r[:], Act.Exp,
                                         scale=-0.5)
                    # nh = hc * rstd (bcast via K=1 matmul) ... then *g+be
                    psb = b2ps.tile([H, BPC], dt.float32)
                    for sl in _chunks(BPC):
                        nc.tensor.matmul(psb[:, sl], onesrow[:, 0:H],
                                         rstd2r[:, sl], start=True, stop=True)
                    rsb = b2.tile([H, BPC], dt.float32)
                    nc.vector.tensor_copy(rsb[:], psb[:])
                    nh2 = b2.tile([H, BPC], dt.float32)
                    nc.vector.tensor_tensor(nh2[:], ps2c[:], rsb[:], Alu.mult)
                    g2 = np.broadcast_to(g21[:, None], (H, 1)).copy()
                    be2 = np.broadcast_to(be21[:, None], (H, 1)).copy()
                    c_g2 = nc.inline_tensor(g2.astype(np.float32), "c_g2")
                    c_be2 = nc.inline_tensor(be2.astype(np.float32), "c_be2")
                    g2_sb = b2.tile([H, 1], dt.float32)
                    nc.sync.dma_start(g2_sb[:], c_g2.ap())
                    be2_sb = b2.tile([H, 1], dt.float32)
                    nc.sync.dma_start(be2_sb[:], c_be2.ap())
                    nc.vector.tensor_scalar(nh2[:], nh2[:], g2_sb[:],
                                            be2_sb[:], Alu.mult, Alu.add)
                    nc.scalar.activation(rl2[:], nh2[:], Act.Relu)
                else:
                    nc.scalar.activation(rl2[:], ps2c[:], Act.Relu)
                psD = b2ps.tile([1, BPC], dt.float32)
                for sl in _chunks(BPC):
                    nc.tensor.matmul(psD[0:1, sl], w22_sb, rl2[:, sl],
                                     start=True, stop=True)
                # xd = sigmoid(rstd2*dots2 + b22)  (fast path)
                #      sigmoid(dots2 + b22)        (affine path: rstd inside)
                xdrow = b2.tile([1, BPC], dt.float32, tag="xdrow")
                if ln_affine:
                    en2a = b2.tile([1, BPC], dt.float32)
                    nc.scalar.activation(en2a[:], psD[0:1, :], Act.Exp,
                                         scale=-1.0, bias=nb22_sb[0:1, :])
                    nc.vector.tensor_scalar(en2a[:], en2a[:], 1.0, None,
                                            Alu.add)
                    nc.vector.reciprocal(xdrow[:], en2a[:])
                else:
                    lnv2 = b2.tile([1, BPC], dt.float32, tag="lnv2")
                    nc.scalar.activation(lnv2[:], psS2[0:1, :], Act.Ln,
                                         scale=1.0 / H, bias=eps_sb[0:1, :])
                    rstd2 = b2.tile([1, BPC], dt.float32, tag="rstd2")
                    nc.scalar.activation(rstd2[:], lnv2[:], Act.Exp,
                                         scale=-0.5)
                    xin2 = b2.tile([1, BPC], dt.float32, tag="xin2")
                    nc.vector.tensor_tensor(xin2[:], psD[0:1, :], rstd2[:],
                                            Alu.mult)
                    en2 = b2.tile([1, BPC], dt.float32, tag="en2")
                    nc.scalar.activation(en2[:], xin2[:], Act.Exp,
                                         scale=-1.0, bias=nb22_sb[0:1, :])
                    nc.vector.tensor_scalar(en2[:], en2[:], 1.0, None, Alu.add)
                    nc.vector.reciprocal(xdrow[:], en2[:])
                # broadcast xd over partitions: K=1 ones matmul
                psX = b2ps.tile([128, BPC], dt.float32)
                for sl in _chunks(BPC):
                    nc.tensor.matmul(psX[:, sl], onesrow[:], xdrow[:, sl],
                                     start=True, stop=True)
                nc.vector.tensor_copy(xdbc_sb[:], psX[:])

            # ================= main loop over groups of 8 b =================
            with tc.tile_pool(name="xt", bufs=6) as xpool, \
                 tc.tile_pool(name="psh", bufs=2, space="PSUM") as pshpool, \
                 tc.tile_pool(name="psx", bufs=5, space="PSUM") as psxpool, \
                 tc.tile_pool(name="big", bufs=2) as bigp, \
                 tc.tile_pool(name="sm", bufs=8) as smp:
                assert NG % 4 == 0
                for p_u in range(unroll * (NG // 4)):
                    p = p_u % (NG // 4)
                    # per-pair staging for 16-wide scalar chain
                    sqs = smp.tile([128, 32], dt.float32, tag="sqs")
                    dots = smp.tile([128, 32], dt.float32, tag="dots")
                    vx = smp.tile([128, 32], dt.float32, tag="vx")
                    nc.vector.tensor_scalar(vx[:],
                                            xdbc_sb[:, p * 32:(p + 1) * 32],
                                            w31, b3 + 1e-5, Alu.mult, Alu.add)
                    pss = []
                    sqv2 = bigp.tile([128, 2048], dt.float32, tag="sqv2")
                    rl2w = bigp.tile([128, 2048], dt.float32, tag="rl2w")
                    xt2w = bigp.tile([128, 2048], dt.float32, tag="xt2w")
                    # -------- phase 1: per-group heavy ops --------
                    for k in range(4):
                        g = 4 * p + k
                        xt = xpool.tile([T + 1, 8 * (N + 1)], dt.float32)
                        nc.sync.dma_start(xt[:],
                                          x_in[:, g * 8:(g + 1) * 8, :])
                        ps_h = pshpool.tile([128, 512], dt.float32)
                        ps_x = psxpool.tile([128, 512], dt.float32)
                        for j in range(8):
                            lhs = xt[:, j * (N + 1): j * (N + 1) + N]
                            nc.tensor.matmul(ps_h[:, j * H:(j + 1) * H],
                                             lhs, RW_sb[:, 0:H], start=True,
                                             stop=True)
                            nc.tensor.matmul(ps_x[:, j * T:(j + 1) * T],
                                             lhs, RW_sb[:, H:2 * H],
                                             start=True, stop=True)
                        hsl = slice(k * 512, (k + 1) * 512)
                        nc.scalar.copy(xt2w[:, hsl], ps_x[:])
                        nc.scalar.square(sqv2[:, hsl], ps_h[:])
                        if not ln_affine:
                            nc.scalar.activation(rl2w[:, hsl], ps_h[:],
                                                 Act.Relu)
                        pss.append(ps_h)

                    # -------- phase 2: pair-wide reductions + rstd --------
                    nc.vector.tensor_reduce(
                        sqs[:], sqv2[:].rearrange("p (g t) -> p g t", g=32),
                        Ax.X, Alu.add)
                    if not ln_affine:
                        dotp2 = bigp.tile([128, 2048], dt.float32, tag="dotp2")
                        w12v2 = w12_sb[:].unsqueeze(1).broadcast_to(
                            [128, 32, H])
                        nc.vector.tensor_tensor(
                            dotp2[:].rearrange("p (g t) -> p g t", g=32),
                            rl2w[:].rearrange("p (g t) -> p g t", g=32),
                            w12v2, Alu.mult)
                        nc.vector.tensor_reduce(
                            dots[:],
                            dotp2[:].rearrange("p (g t) -> p g t", g=32),
                            Ax.X, Alu.add)
                    lnv = smp.tile([128, 32], dt.float32, tag="lnv")
                    nc.scalar.activation(lnv[:], sqs[:], Act.Ln,
                                         scale=1.0 / H, bias=eps_sb[:])
                    rstd = smp.tile([128, 32], dt.float32, tag="rstd")
                    nc.scalar.activation(rstd[:], lnv[:], Act.Exp, scale=-0.5)
                    if ln_affine:
                        for k in range(4):
                            ps = pss[k]
                            nh = bigp.tile([128, 512], dt.float32, tag="nh")
                            nh3 = nh[:].rearrange("p (g t) -> p g t", g=8)
                            rst_b = rstd[:, k * 8:(k + 1) * 8].unsqueeze(
                                2).broadcast_to([128, 8, H])
                            nc.vector.tensor_tensor(
                                nh3,
                                ps[:].rearrange("p (g t) -> p g t", g=8),
                                rst_b, Alu.mult)
                            g_b = g11_sb[:].unsqueeze(1).broadcast_to(
                                [128, 8, H])
                            be_b = be11_sb[:].unsqueeze(1).broadcast_to(
                                [128, 8, H])
                            nc.vector.tensor_tensor(nh3, nh3, g_b, Alu.mult)
                            nc.vector.tensor_tensor(nh3, nh3, be_b, Alu.add)
                            rl = bigp.tile([128, 512], dt.float32, tag="rl")
                            nc.vector.tensor_scalar(rl[:], nh[:], 0.0, None,
                                                    Alu.max)
                            dotp = bigp.tile([128, 512], dt.float32,
                                             tag="dotp")
                            w12v = w12_sb[:].unsqueeze(1).broadcast_to(
                                [128, 8, H])
                            nc.gpsimd.tensor_tensor(
                                dotp[:].rearrange("p (g t) -> p g t", g=8),
                                rl[:].rearrange("p (g t) -> p g t", g=8),
                                w12v, Alu.mult)
                            nc.vector.tensor_reduce(
                                dots[:, k * 8:(k + 1) * 8],
                                dotp[:].rearrange("p (g t) -> p g t", g=8),
                                Ax.X, Alu.add)

                    # -------- phase 3: 16-wide scalar chain --------
                    xin = smp.tile([128, 32], dt.float32, tag="xin")
                    if ln_affine:
                        nc.vector.tensor_copy(xin[:], dots[:])
                    else:
                        nc.vector.tensor_tensor(xin[:], dots[:], rstd[:],
                                                Alu.mult)
                    exu = smp.tile([128, 32], dt.float32, tag="exu")
                    nc.scalar.activation(exu[:], xin[:], Act.Exp, scale=-1.0,
                                         bias=nb12_sb[:])
                    nc.vector.tensor_scalar(exu[:], exu[:], 1.0, None, Alu.add)
                    xu = smp.tile([128, 32], dt.float32, tag="xu")
                    nc.vector.reciprocal(xu[:], exu[:])
                    # v + 1e-5 = w30*xu + (w31*xd + b3 + 1e-5)
                    v1 = smp.tile([128, 32], dt.float32, tag="v1")
                    nc.vector.scalar_tensor_tensor(v1[:], xu[:], w30, vx[:],
                                                   Alu.mult, Alu.add)
                    rr = smp.tile([128, 32], dt.float32, tag="rr")
                    nc.vector.reciprocal(rr[:], v1[:])
                    fden = smp.tile([128, 32], dt.float32, tag="fden")
                    nc.vector.tensor_scalar(fden[:], rr[:], 50.0 * alpha, 1.0,
                                            Alu.mult, Alu.add)
                    F = smp.tile([128, 32], dt.float32, tag="F")
                    nc.vector.reciprocal(F[:], fden[:])
                    q = smp.tile([128, 32], dt.float32, tag="q")
                    nc.vector.tensor_scalar(q[:], F[:], -1.0, 1.0, Alu.mult,
                                            Alu.add)
                    if general_tail:
                        y5 = smp.tile([128, 32], dt.float32, tag="y5")
                        nc.vector.tensor_scalar(y5[:], rr[:], 5.0, 0.5,
                                                Alu.mult, Alu.add)
                        yi = smp.tile([128, 32], dt.int32, tag="yi")
                        nc.vector.tensor_copy(yi[:], y5[:])
                        yf = smp.tile([128, 32], dt.float32, tag="yf")
                        nc.vector.tensor_copy(yf[:], yi[:])
                        Tc = smp.tile([128, 32], dt.float32, tag="Tc")
                        nc.vector.tensor_scalar(Tc[:], yf[:], 0.0, 63.0,
                                                Alu.max, Alu.min)
                        mst = smp.tile([128, 32], dt.float32, tag="mst")
                        nc.vector.tensor_scalar(mst[:], Tc[:], -1.0, 63.0,
                                                Alu.mult, Alu.add)
                        men = smp.tile([128, 32], dt.float32, tag="men")
                        nc.vector.tensor_scalar(men[:], Tc[:], -1.0, 64.0,
                                                Alu.mult, Alu.add)

                    # ---- phase 4: pair-wide a-build + scan + accumulate ----
                    a2 = bigp.tile([128, 2048], dt.float32, tag="a2")
                    qb2 = q[:].rearrange("p (a g) -> p a g", a=4).unsqueeze(
                        3).broadcast_to([128, 4, 8, T])
                    t0v2 = t0m_sb[:].rearrange(
                        "p (g t) -> p g t", g=8).unsqueeze(1).broadcast_to(
                        [128, 4, 8, T])
                    nc.vector.tensor_tensor(
                        a2[:].rearrange("p (a g t) -> p a g t", a=4, g=8),
                        qb2, t0v2, Alu.mult)
                    s2 = bigp.tile([128, 2048], dt.float32, tag="s2")
                    nc.vector.tensor_tensor_scan(
                        s2[:], a2[:], xt2w[:], 0.0, Alu.mult, Alu.add)
                    accs = acc_sb[:, p * 32:(p + 1) * 32]
                    if general_tail:
                        sstar = smp.tile([128, 32], dt.float32, tag="sstar")
                        junk = bigp.tile([128, 64], dt.float32, tag="junk")
                        for j in range(16):
                            nc.vector.tensor_mask_reduce(
                                junk[:], s2[:, j * T:(j + 1) * T],
                                mst[:, j:j + 1], men[:, j:j + 1], 1.0,
                                -3.0e38, Alu.max,
                                accum_out=sstar[:, j:j + 1])
                        nc.vector.tensor_tensor(accs, sstar[:], F[:],
                                                Alu.mult)
                    else:
                        slast = s2[:].rearrange(
                            "p (g t) -> p g t", g=32)[:, :, T - 1]
                        nc.vector.tensor_tensor(accs, slast, F[:], Alu.mult)

                # ---- final: pred[b] = sum over partitions of acc ----
                with tc.tile_pool(name="fin", bufs=1) as fin, \
                     tc.tile_pool(name="finps", bufs=1, space="PSUM") as fps:
                    po = fps.tile([1, BPC], dt.float32)
                    for sl in _chunks(BPC):
                        nc.tensor.matmul(po[0:1, sl], ones_sb[:],
                                         acc_sb[:, sl], start=True, stop=True)
                    pred = fin.tile([1, BPC], dt.float32)
                    nc.vector.tensor_copy(pred[:], po[0:1, :])
                    nc.sync.dma_start(
                        y_out.rearrange("b one -> one b"), pred[:])

    # Force all activations onto the one table set that contains every
    # function we use (Relu/Square/Ln/Exp/Copy/Identity), so the compiled
    # stream has a single ACT table load instead of per-group thrash.
    # The pass picks the first listed set containing each function; ids must
    # stay aligned with act_info.json order, so empty out the other sets.
    import types
    from concourse.hw_specs import get_activation_tables
    import concourse._compat as _cc
    orig_tables = list(get_activation_tables(nc.m.arch).items())
    patched_tables = [
        (name, s if name == "natural_log_exp_and_others" else set())
        for name, s in orig_tables
    ]
    import bass_rust as _bass_rust_mod

    def _patched_act_loads(self):
        has_activation = any(
            type(i).__name__ == "InstActivation"
            for b in self.main_func.blocks
            for i in b.instructions
        )
        if not has_activation:
            return
        _bass_rust_mod.insert_act_table_loads(self, patched_tables)

    nc.insert_act_table_loads = types.MethodType(_patched_act_loads, nc)

    nc.compile()
    built = _Built()
    built.nc = nc
    built.BPC = BPC
    return built




def _build(weights, BPC, ln_affine, general_tail, unroll=1):
    """Poly fast path when valid, else the scan kernel."""
    if (not ln_affine) and (not general_tail) and poly_gate(weights):
        return build_poly(weights, BPC, unroll=unroll)
    return _build_scan(weights, BPC, ln_affine, general_tail, unroll=unroll)


_CACHE = {}


def _get_built(weights, BPC, ln_affine, general_tail):
    full_key = (BPC, ln_affine, general_tail,
                b"".join(_np32(weights[k]).tobytes() for k in sorted(weights)))
    if full_key not in _CACHE:
        _CACHE[full_key] = _build(weights, BPC, ln_affine, general_tail)
    return _CACHE[full_key]


def _fold_weights(inputs):
    mean = float(np.asarray(inputs["x_mean"]))
    std = float(np.asarray(inputs["x_std"]))
    W11r = _np32(inputs["W11"])
    W21r = _np32(inputs["W21"])
    w = {
        "W11": W11r / std,
        "b11": _np32(inputs["b11"]) - (mean / std) * W11r.sum(0),
        "W21": W21r / std,
        "b21": _np32(inputs["b21"]) - (mean / std) * W21r.sum(0),
        "W12": _np32(inputs["W12"])[:, 0],
        "b12": float(np.asarray(inputs["b12"])[0]),
        "W22": _np32(inputs["W22"])[:, 0],
        "b22": float(np.asarray(inputs["b22"])[0]),
        "g11": _np32(inputs["g11"]), "be11": _np32(inputs["be11"]),
        "g21": _np32(inputs["g21"]), "be21": _np32(inputs["be21"]),
        "w30": float(np.asarray(inputs["W3"])[0, 0]),
        "w31": float(np.asarray(inputs["W3"])[1, 0]),
        "b3": float(np.asarray(inputs["b3"])[0]),
        "alpha": float(np.asarray(inputs["alpha"])[0]),
    }
    return w


def _tail_is_degenerate(w):
    """True iff v+1e-5 is provably inside (-10+m, -m) for all sigmoid outputs,
    which forces round(Tv/10) <= -1 -> T_idx clamps to 0 -> Ln == 64."""
    lo = w["b3"] + 1e-5 + min(w["w30"], 0.0) + min(w["w31"], 0.0)
    hi = w["b3"] + 1e-5 + max(w["w30"], 0.0) + max(w["w31"], 0.0)
    m = 1e-3
    return (lo > -10.0 + m) and (hi < -m) and w["alpha"] >= 0.0


def _use_poly(w, ln_affine, general_tail):
    return (not ln_affine) and (not general_tail) and poly_gate(w)


def make_in_maps(x, poly):
    """Per-core input staging. poly: bf16 t-major 2D x + f32 down channel.
    scan fallback: f32 t-major padded x + f32 down channel."""
    BPC = B // N_CORES
    in_maps = []
    bf = mybir.dt.np(dt.bfloat16)
    for c in range(N_CORES):
        xs = x[c * BPC:(c + 1) * BPC]          # [BPC, T, N+1]
        xp = np.empty((T + 1, BPC, N + 1), np.float32)
        xp[:T] = xs.transpose(1, 0, 2)
        xp[T] = 1.0
        d = np.ascontiguousarray(xp[:, :, N])
        if poly:
            in_maps.append({"x": np.ascontiguousarray(
                xp.astype(bf).reshape(T + 1, BPC * (N + 1))),
                "d": d.astype(bf)})
        else:
            in_maps.append({"x": xp, "d": d})
    return in_maps


def kernel(**inputs) -> np.ndarray:
    x = _np32(inputs["x"])
    assert x.shape == (B, T, N + 1)
    w = _fold_weights(inputs)
    ln_affine = not (np.all(w["g11"] == 1.0) and np.all(w["be11"] == 0.0)
                     and np.all(w["g21"] == 1.0) and np.all(w["be21"] == 0.0))
    general_tail = not _tail_is_degenerate(w)
    BPC = B // N_CORES
    built = _get_built(w, BPC, ln_affine, general_tail)
    in_maps = make_in_maps(x, _use_poly(w, ln_affine, general_tail))
    res = run_bass_kernel_spmd(built.nc, in_maps, list(range(N_CORES)))
    out = np.concatenate([r["y"] for r in res.results], axis=0)
    return out.astype(np.float32)


if __name__ == "__main__":
    print("kernel module ok")

